# revision 1
# baseline (speedup 1.0000x reference)
"""Trainium2 Bass kernel for a 2-layer de-stationary-attention transformer.

Model (per reference):
  L=2 layers of: x += DSAttn(x); x = LN1(x); x = LN2(x + FFN(x)); then
  final LN + output projection Wp.
  DSAttn: softmax(scale * (Q K^T * tau + delta)) V with per-batch tau,
  per-(batch, key) delta.

Shapes: B=16, S=512, D=1024, H=16 heads (dh=64), F=4096.

Sharding: data-parallel over batch across 8 NeuronCores (2 batches/core),
weights replicated. No collectives.

Per-core layout strategy (all on-chip between DRAM load and store):
  - Residual stream x kept FEATURE-major: 8 SBUF tiles [128 (d), 1024 (tok)].
    All matmuls contract over the partition dim, so projections consume x
    directly as the moving operand.
  - Scores are computed pre-transposed S^T[s, l] = K_slice^T Q_slice so the
    softmax denominator direction lands on the free dim of the AV matmul,
    and tau/delta fold into the ScalarE exp as scale/bias (both
    per-partition after the transpose).
  - V is produced token-major [tok, dout] (x as the stationary operand) with
    a ones-column appended per head, so each AV matmul also emits the
    softmax denominator row for free.
  - Per-token (free-dim) normalizers (softmax recip, LN mean/rstd) are
    broadcast across partitions with K=1 matmuls into PSUM.
  - LayerNorm column sums via ones-vector matmuls (partition reduction on
    the PE), then a 3-op normalize (2 DVE + 1 ACT with per-partition
    gain/bias).
  - Matmuls run in float32r (reduced-precision fp32, full PE rate at
    N>=256), accumulating in fp32 PSUM.

Weights are pre-transposed/pre-tiled on the host so every DMA is a large
contiguous read. Host also pre-transposes x to feature-major and folds the
1/sqrt(dh) softmax scale into tau/delta.
"""

import sys

if "/opt/trn_rl_repo" not in sys.path:
    sys.path.insert(0, "/opt/trn_rl_repo")

import numpy as np

import concourse.bass as bass
import concourse.bacc as bacc
import concourse.tile as tile
import concourse.mybir as mybir
from concourse import bass_utils

# Model dims
L, D, H, F = 2, 1024, 16, 4096
B, S = 16, 512
DH = D // H  # 64
NCORES = 8
BPC = B // NCORES  # batches per core
P = 128
NDT = D // P        # 8 d-tiles
NST = S // P        # 4 s-tiles per batch
NTOK = BPC * S      # 1024 tokens per core
NHP = H // 2        # 8 head pairs
FC = 512            # FFN f-chunk size
NFC = F // FC       # 8 chunks
VW = DH + 1         # 65: value width per head incl. ones column
EPS = 1e-5

F32 = mybir.dt.float32
MM = mybir.dt.float32r  # matmul operand dtype
AF = mybir.ActivationFunctionType
ALU = mybir.AluOpType

_CACHE: dict = {}
import os
KPART = os.environ.get("KPART", "full")  # full | attn | ffn | noln
KGELU = os.environ.get("KGELU", "gelu")  # CoreSim lacks Gelu; "id" to swap


def _build(reps: int):
    key = (reps, KPART, KGELU)
    if key in _CACHE:
        return _CACHE[key]

    nc = bacc.Bacc("TRN2", target_bir_lowering=False, debug=False,
                   num_devices=NCORES)

    # ---- DRAM tensors (per-core shapes) ----
    # matmul-feeding tensors are float32r (same bits as f32)
    x_d = nc.dram_tensor("x_fm", (BPC, D, S), MM, kind="ExternalInput")
    wq_d = nc.dram_tensor("wq_t", (L, NDT, D, P), MM, kind="ExternalInput")
    wk_d = nc.dram_tensor("wk_t", (L, NDT, D, P), MM, kind="ExternalInput")
    wv_d = nc.dram_tensor("wv_t", (L, D, D), MM, kind="ExternalInput")
    wo_d = nc.dram_tensor("wo_t", (L, NDT, D, P), MM, kind="ExternalInput")
    w1_d = nc.dram_tensor("w1_t", (L, NFC, NDT, P, FC), MM, kind="ExternalInput")
    w2_d = nc.dram_tensor("w2_t", (L, F, D), MM, kind="ExternalInput")
    wp_d = nc.dram_tensor("wp_t", (NDT, D, P), MM, kind="ExternalInput")
    bv_d = nc.dram_tensor("bv", (L, D), MM, kind="ExternalInput")

    bq_d = nc.dram_tensor("bq", (L, D), F32, kind="ExternalInput")
    bk_d = nc.dram_tensor("bk", (L, D), F32, kind="ExternalInput")
    bo_d = nc.dram_tensor("bo", (L, D), F32, kind="ExternalInput")
    b1_d = nc.dram_tensor("b1", (L, F), F32, kind="ExternalInput")
    b2_d = nc.dram_tensor("b2", (L, D), F32, kind="ExternalInput")
    g1_d = nc.dram_tensor("g1", (L, D), F32, kind="ExternalInput")
    be1_d = nc.dram_tensor("be1", (L, D), F32, kind="ExternalInput")
    g2_d = nc.dram_tensor("g2", (L, D), F32, kind="ExternalInput")
    be2_d = nc.dram_tensor("be2", (L, D), F32, kind="ExternalInput")
    gf_d = nc.dram_tensor("gf", (D,), F32, kind="ExternalInput")
    bf_d = nc.dram_tensor("bf", (D,), F32, kind="ExternalInput")
    bp_d = nc.dram_tensor("bp", (D,), F32, kind="ExternalInput")
    ident_d = nc.dram_tensor("ident", (P, P), MM, kind="ExternalInput")
    stau_d = nc.dram_tensor("sc_tau", (BPC, P), F32, kind="ExternalInput")
    sdel_d = nc.dram_tensor("sc_delta", (BPC, S), F32, kind="ExternalInput")

    out_d = nc.dram_tensor("out_fm", (BPC, D, S), F32, kind="ExternalOutput")

    with tile.TileContext(nc) as tc:
        _emit(nc, tc, reps, locals())

    nc.compile()
    _CACHE[key] = nc
    return nc


def _emit(nc, tc, reps, d):
    x_d, wq_d, wk_d, wv_d, wo_d, w1_d, w2_d, wp_d = (
        d["x_d"], d["wq_d"], d["wk_d"], d["wv_d"], d["wo_d"], d["w1_d"],
        d["w2_d"], d["wp_d"])
    bv_d, bq_d, bk_d, bo_d, b1_d, b2_d = (
        d["bv_d"], d["bq_d"], d["bk_d"], d["bo_d"], d["b1_d"], d["b2_d"])
    g1_d, be1_d, g2_d, be2_d, gf_d, bf_d, bp_d = (
        d["g1_d"], d["be1_d"], d["g2_d"], d["be2_d"], d["gf_d"], d["bf_d"],
        d["bp_d"])
    stau_d, sdel_d, out_d = d["stau_d"], d["sdel_d"], d["out_d"]
    ident_d = d["ident_d"]

    from contextlib import ExitStack
    ctx = ExitStack()
    # Static SBUF budget is ~192KB/partition; non-overlapping-lifetime
    # buffers share pool tags (o/y, v/h, qk/w1, out/tmp).
    singles = ctx.enter_context(tc.tile_pool(name="singles", bufs=1))
    xpool = ctx.enter_context(tc.tile_pool(name="xpool", bufs=1))
    vhpool = ctx.enter_context(tc.tile_pool(name="vhpool", bufs=1))
    oypool = ctx.enter_context(tc.tile_pool(name="oypool", bufs=1))
    qw1pool = ctx.enter_context(tc.tile_pool(name="qw1pool", bufs=5))
    wpool = ctx.enter_context(tc.tile_pool(name="wpool", bufs=8))
    epool = ctx.enter_context(tc.tile_pool(name="epool", bufs=8))
    tmppool = ctx.enter_context(tc.tile_pool(name="tmppool", bufs=4))
    rowpool = ctx.enter_context(tc.tile_pool(name="rowpool", bufs=5))
    psA = ctx.enter_context(tc.tile_pool(name="psA", bufs=4, space="PSUM"))
    psX = ctx.enter_context(tc.tile_pool(name="psX", bufs=4, space="PSUM"))

    # ---- constants / params (loaded once, outside the reps loop) ----
    # memset cannot write float32r; bounce constants through f32 + ACT copy
    ones_col_f = singles.tile([P, 1], F32)
    nc.vector.memset(ones_col_f, 1.0)
    ones_col = singles.tile([P, 1], MM)
    nc.scalar.activation(ones_col, ones_col_f, AF.Copy)
    ones_row_f = singles.tile([1, P], F32)
    nc.vector.memset(ones_row_f, 1.0)
    ones_row = singles.tile([1, P], MM)
    nc.scalar.activation(ones_row, ones_row_f, AF.Copy)
    onesH_f = singles.tile([P, H], F32)
    nc.vector.memset(onesH_f, 1.0)
    eps_row = singles.tile([1, 1], F32)
    nc.vector.memset(eps_row, EPS)
    ident = singles.tile([P, P], MM)
    nc.sync.dma_start(ident, ident_d.ap())

    def load_cols(dram_row, ncols):
        # [ncols*P] DRAM vector -> [P, ncols] SBUF (partition-major)
        t = singles.tile([P, ncols], dram_row.dtype,
                         name=f"prm_{dram_row.tensor.name}_{nc.next_id()}")
        nc.sync.dma_start(t, dram_row.rearrange("(t p) -> p t", p=P))
        return t

    bq_sb, bk_sb, bo_sb, b2_sb = [], [], [], []
    g1_sb, be1_sb, g2_sb, be2_sb, b1_sb = [], [], [], [], []
    bv_sb = singles.tile([1, L * D], MM)
    for l in range(L):
        bq_sb.append(load_cols(bq_d[l], NDT))
        bk_sb.append(load_cols(bk_d[l], NDT))
        bo_sb.append(load_cols(bo_d[l], NDT))
        b2_sb.append(load_cols(b2_d[l], NDT))
        g1_sb.append(load_cols(g1_d[l], NDT))
        be1_sb.append(load_cols(be1_d[l], NDT))
        g2_sb.append(load_cols(g2_d[l], NDT))
        be2_sb.append(load_cols(be2_d[l], NDT))
        b1_sb.append(load_cols(b1_d[l], F // P))
        nc.sync.dma_start(bv_sb[:, l * D:(l + 1) * D], bv_d[l][None, :])
    gf_sb = load_cols(gf_d.ap(), NDT)
    bf_sb = load_cols(bf_d.ap(), NDT)
    bp_sb = load_cols(bp_d.ap(), NDT)
    stau_sb = singles.tile([P, BPC], F32)
    nc.sync.dma_start(stau_sb, stau_d.ap().rearrange("b p -> p b"))
    sdel_sb = singles.tile([P, BPC * NST], F32)
    nc.sync.dma_start(sdel_sb.rearrange("p (b t) -> p b t", b=BPC),
                      sdel_d.ap().rearrange("b (t p) -> p b t", p=P))

    def body(_i=None):
        # ---- load x (feature-major) ----
        x_sb = []
        for dt in range(NDT):
            xt = xpool.tile([P, NTOK], MM, name=f"x_{dt}", tag=f"x_{dt}")
            for b in range(BPC):
                nc.sync.dma_start(
                    xt[:, b * S:(b + 1) * S],
                    x_d[b, dt * P:(dt + 1) * P, :])
            x_sb.append(xt)

        def ln(src, dst, g_t, be_t):
            """LayerNorm over d (partitions): src/dst are lists of 8 fm
            tiles; g_t/be_t are [P, NDT] per-partition param tiles.
            Stats for both batches first (PE colsums + short row chains),
            then per-tile normalize: PE re-streams x into PSUM with the
            negated mean added (identity matmul + K=1 ones x mean_n), so
            the per-tile cost is one DVE mul + one ACT affine."""
            rows_rs, rows_nm = [], []
            for b in range(BPC):
                cs = slice(b * S, (b + 1) * S)
                ps_s = psA.tile([1, S], F32, name="ps_s", tag="ps")
                for dt in range(NDT):
                    nc.tensor.matmul(ps_s, ones_col, src[dt][:, cs],
                                     start=(dt == 0), stop=(dt == NDT - 1))
                ps_q = psA.tile([1, S], F32, name="ps_q", tag="ps")
                for dt in range(NDT):
                    sq = tmppool.tile([P, S], MM, name="sq", tag="tmp")
                    nc.scalar.activation(sq, src[dt][:, cs], AF.Square)
                    nc.tensor.matmul(ps_q, ones_col, sq,
                                     start=(dt == 0), stop=(dt == NDT - 1))
                mean_n = rowpool.tile([1, S], MM, name="mean_n", tag="row")
                nc.vector.tensor_scalar(mean_n, ps_s, -1.0 / D, None, ALU.mult)
                var = rowpool.tile([1, S], F32, name="var", tag="row")
                nc.vector.tensor_scalar(var, ps_q, 1.0 / D, None, ALU.mult)
                m2 = rowpool.tile([1, S], F32, name="m2", tag="row")
                nc.vector.tensor_mul(m2, mean_n, mean_n)
                nc.vector.tensor_sub(var, var, m2)
                sd = rowpool.tile([1, S], F32, name="sd", tag="row")
                nc.scalar.activation(sd, var, AF.Sqrt, bias=eps_row)
                rs_r = rowpool.tile([1, S], MM, name="rs_r", tag="row")
                with nc.allow_low_precision(reason="f32r rows feed matmuls"):
                    nc.vector.reciprocal(rs_r, sd)
                rows_rs.append(rs_r)
                rows_nm.append(mean_n)
            for b in range(BPC):
                cs = slice(b * S, (b + 1) * S)
                pb_rs = psX.tile([P, S], F32, name="pb_rs", tag="px")
                nc.tensor.matmul(pb_rs, ones_row, rows_rs[b])
                rs_sb = tmppool.tile([P, S], F32, name="rs_sb", tag="tmp")
                nc.scalar.activation(rs_sb, pb_rs, AF.Copy)
                for dt in range(NDT):
                    pc = psX.tile([P, S], F32, name="pc", tag="px")
                    nc.tensor.matmul(pc, ident, src[dt][:, cs],
                                     start=True, stop=False)
                    nc.tensor.matmul(pc, ones_row, rows_nm[b],
                                     start=False, stop=True)
                    t1 = tmppool.tile([P, S], F32, name="t1", tag="tmp")
                    nc.vector.tensor_mul(t1, pc, rs_sb)
                    nc.scalar.activation(dst[dt][:, cs], t1, AF.Identity,
                                         scale=g_t[:, dt:dt + 1],
                                         bias=be_t[:, dt:dt + 1])

        def attn_phase(l):
            # ================= attention =================
            # ---- V (token-major, ones col per head) ----
            wv_sb = []
            for dt in range(NDT):
                wt = wpool.tile([P, D], MM, name=f"wv_{dt}", tag="w")
                nc.sync.dma_start(wt, wv_d[l, dt * P:(dt + 1) * P, :])
                wv_sb.append(wt)
            v_sb = []
            for tt in range(NDT):
                vt = vhpool.tile([P, H * VW], MM, name=f"v_{tt}", tag=f"vh_{tt}")
                nc.scalar.activation(
                    vt.rearrange("p (h e) -> p h e", e=VW)[:, :, DH:DH + 1],
                    onesH_f.rearrange("p (h e) -> p h e", e=1), AF.Copy)
                v_sb.append(vt)
            for tt in range(NDT):
                ts = slice(tt * P, (tt + 1) * P)
                for nh in range(2):
                    ps = psA.tile([P, S], F32, name="ps_v", tag="ps")
                    for dt in range(NDT):
                        nc.tensor.matmul(
                            ps, x_sb[dt][:, ts],
                            wv_sb[dt][:, nh * 512:(nh + 1) * 512],
                            start=(dt == 0), stop=False)
                    nc.tensor.matmul(
                        ps, ones_row[:, :P],
                        bv_sb[:, l * D + nh * 512: l * D + (nh + 1) * 512],
                        start=False, stop=True)
                    dstv = v_sb[tt][:, nh * 8 * VW:(nh + 1) * 8 * VW]
                    nc.scalar.activation(
                        dstv.rearrange("p (h e) -> p h e", e=VW)[:, :, 0:DH],
                        ps.rearrange("p (h e) -> p h e", e=DH),
                        AF.Copy)
            # ---- per head pair: Q, K, scores, softmax, AV ----
            # Software-pipelined with a one-stage skew: head i+1's
            # scores+exp are emitted before head i's AV/normalize, so the
            # in-order PE queue never stalls waiting on ACT(exp)/DVE(recip).
            o_sb = []
            pending = []

            def s2_flush():
                if pending:
                    pending.pop(0)()

            for hp in range(NHP):
                wq_p = wpool.tile([P, NDT, P], MM, name="wq_p", tag="w")
                nc.sync.dma_start(
                    wq_p, wq_d[l, hp].rearrange("(t p) m -> p t m", p=P))
                wk_p = wpool.tile([P, NDT, P], MM, name="wk_p", tag="w")
                nc.sync.dma_start(
                    wk_p, wk_d[l, hp].rearrange("(t p) m -> p t m", p=P))
                q_p = qw1pool.tile([P, NTOK], MM, name="q_p", tag="qw1")
                k_p = qw1pool.tile([P, NTOK], MM, name="k_p", tag="qw1")
                for b in range(BPC):
                    cs = slice(b * S, (b + 1) * S)
                    for wt, dst, bias in ((wq_p, q_p, bq_sb[l]),
                                          (wk_p, k_p, bk_sb[l])):
                        ps = psA.tile([P, S], F32, name="ps_qk", tag="ps")
                        for dt in range(NDT):
                            nc.tensor.matmul(ps, wt[:, dt, :],
                                             x_sb[dt][:, cs],
                                             start=(dt == 0),
                                             stop=(dt == NDT - 1))
                        nc.scalar.activation(dst[:, cs], ps, AF.Identity,
                                             bias=bias[:, hp:hp + 1])
                ot = oypool.tile([P, NTOK], MM, name=f"o_{hp}", tag=f"oy_{hp}")
                o_sb.append(ot)
                if KPART == "proj":
                    nc.scalar.activation(ot, q_p, AF.Copy)
                    continue
                for b in range(BPC):
                    cs = slice(b * S, (b + 1) * S)
                    for lh in range(2):
                        h = hp * 2 + lh
                        rb = lh * DH
                        rsl = slice(rb, rb + DH)
                        ets = []
                        for st in range(NST):
                            ps = psA.tile([P, S], F32, name="ps_sc", tag="ps")
                            nc.tensor.matmul(
                                ps,
                                k_p[rsl, b * S + st * P: b * S + (st + 1) * P],
                                q_p[rsl, cs])
                            et = epool.tile([P, S], MM, name="et", tag="e")
                            if KPART == "noexp":
                                nc.scalar.activation(et, ps, AF.Exp)
                            else:
                                nc.scalar.activation(
                                    et, ps, AF.Exp,
                                    scale=stau_sb[:, b:b + 1],
                                    bias=sdel_sb[:, b * NST + st:
                                                 b * NST + st + 1])
                            ets.append(et)

                        def s2(ets=ets, ot=ot, cs=cs, h=h, b=b, rsl=rsl):
                            po = psX.tile([VW, S], F32, name="po", tag="px")
                            for st in range(NST):
                                nc.tensor.matmul(
                                    po,
                                    v_sb[b * NST + st][:, h * VW:(h + 1) * VW],
                                    ets[st], start=(st == 0),
                                    stop=(st == NST - 1))
                            if KPART == "nosm":
                                nc.scalar.activation(ot[rsl, cs], po[:DH, :],
                                                     AF.Copy)
                                return
                            den_r = rowpool.tile([1, S], MM, name="den_r",
                                                 tag="row")
                            with nc.allow_low_precision(
                                    reason="f32r rows feed matmuls"):
                                nc.vector.reciprocal(den_r, po[DH:DH + 1, :])
                            pb = psX.tile([P, S], F32, name="pb_at", tag="px")
                            nc.tensor.matmul(pb[:DH, :], ones_row[:, :DH],
                                             den_r)
                            onum = tmppool.tile([P, S], F32, name="onum",
                                                tag="tmp")
                            nc.scalar.activation(onum[:DH, :], po[:DH, :],
                                                 AF.Copy)
                            nc.vector.tensor_mul(ot[rsl, cs], onum[:DH, :],
                                                 pb[:DH, :])

                        pending.append(s2)
                        if len(pending) > 1:
                            s2_flush()
            while pending:
                s2_flush()
            # ---- Wo projection + residual into x ----
            for dto in range(NDT):
                wo_p = wpool.tile([P, NDT, P], MM, name="wo_p", tag="w")
                nc.sync.dma_start(
                    wo_p, wo_d[l, dto].rearrange("(t p) m -> p t m", p=P))
                for b in range(BPC):
                    cs = slice(b * S, (b + 1) * S)
                    ps = psA.tile([P, S], F32, name="ps_wo", tag="ps")
                    for dt in range(NDT):
                        nc.tensor.matmul(ps, wo_p[:, dt, :], o_sb[dt][:, cs],
                                         start=(dt == 0), stop=(dt == NDT - 1))
                    t2 = tmppool.tile([P, S], F32, name="t2", tag="tmp")
                    nc.scalar.activation(t2, ps, AF.Identity,
                                         bias=bo_sb[l][:, dto:dto + 1])
                    nc.vector.tensor_add(x_sb[dto][:, cs], x_sb[dto][:, cs],
                                         t2)

        def ffn_phase(l):
            # ================= FFN =================
            y_sb = []
            for dt in range(NDT):
                yt = oypool.tile([P, NTOK], MM, name=f"y_{dt}", tag=f"oy_{dt}")
                y_sb.append(yt)
            # Software-pipelined chunks: chunk c+1's h-matmuls are emitted
            # before chunk c's y-matmuls (h tiles alternate tag groups) so
            # the PE never stalls on the gelu eviction tail.
            def h_block(fc):
                w1_sb = []
                for j in range(NDT // 2):
                    wt = qw1pool.tile([P, 2, FC], MM, name="w1c", tag="qw1")
                    nc.sync.dma_start(
                        wt, w1_d[l, fc, 2 * j:2 * j + 2].rearrange(
                            "d p f -> p d f"))
                    w1_sb.append(wt)
                h_sb = []
                for ft in range(FC // P):
                    ht = vhpool.tile([P, NTOK], MM, name="htile",
                                     tag=f"vh_{(fc % 2) * 4 + ft}")
                    h_sb.append(ht)
                    gfi = fc * (FC // P) + ft
                    for b in range(BPC):
                        cs = slice(b * S, (b + 1) * S)
                        ps = psA.tile([P, S], F32, name="ps_h", tag="ps")
                        for dt in range(NDT):
                            nc.tensor.matmul(
                                ps,
                                w1_sb[dt // 2][:, dt % 2,
                                               ft * P:(ft + 1) * P],
                                x_sb[dt][:, cs],
                                start=(dt == 0), stop=(dt == NDT - 1))
                        gelu_f = AF.Gelu if KGELU == "gelu" else AF.Identity
                        nc.scalar.activation(ht[:, cs], ps, gelu_f,
                                             bias=b1_sb[l][:, gfi:gfi + 1])
                return h_sb

            def y_block(fc, h_sb):
                w2_sb = []
                for i in range(FC // P):
                    wt = wpool.tile([P, D], MM, name="w2c", tag="w")
                    r0 = fc * FC + i * P
                    nc.sync.dma_start(wt, w2_d[l, r0:r0 + P, :])
                    w2_sb.append(wt)
                for dt in range(NDT):
                    for b in range(BPC):
                        cs = slice(b * S, (b + 1) * S)
                        ps = psA.tile([P, S], F32, name="ps_y", tag="ps")
                        for ft in range(FC // P):
                            nc.tensor.matmul(
                                ps, w2_sb[ft][:, dt * P:(dt + 1) * P],
                                h_sb[ft][:, cs],
                                start=(ft == 0), stop=(ft == FC // P - 1))
                        if fc == 0:
                            nc.scalar.activation(y_sb[dt][:, cs], ps,
                                                 AF.Identity,
                                                 bias=b2_sb[l][:, dt:dt + 1])
                        else:
                            nc.vector.tensor_add(y_sb[dt][:, cs], ps,
                                                 y_sb[dt][:, cs])

            prev = None
            for fc in range(NFC):
                h_now = h_block(fc)
                if prev is not None:
                    y_block(fc - 1, prev)
                prev = h_now
            y_block(NFC - 1, prev)
            # residual r2 = x_ln1 + y (into y); LN2(y) -> x
            for dt in range(NDT):
                nc.vector.tensor_add(y_sb[dt], y_sb[dt], x_sb[dt])
            if KPART == "full":
                ln(y_sb, x_sb, g2_sb[l], be2_sb[l])

        ATTN_LIKE = ("full", "attn", "noln", "proj", "noexp", "nosm")
        for l in range(L):
            if KPART in ATTN_LIKE:
                attn_phase(l)
            if KPART == "full":
                ln(x_sb, x_sb, g1_sb[l], be1_sb[l])
            if KPART in ("full", "ffn", "noln"):
                ffn_phase(l)

        # ---- final LN + Wp ----
        if KPART == "full":
            ln(x_sb, x_sb, gf_sb, bf_sb)
        for dto in range(NDT):
            wp_p = wpool.tile([P, NDT, P], MM, name="wp_p", tag="w")
            nc.sync.dma_start(wp_p,
                              wp_d[dto].rearrange("(t p) m -> p t m", p=P))
            for b in range(BPC):
                cs = slice(b * S, (b + 1) * S)
                ps = psA.tile([P, S], F32, name="ps_wp", tag="ps")
                for dt in range(NDT):
                    nc.tensor.matmul(ps, wp_p[:, dt, :], x_sb[dt][:, cs],
                                     start=(dt == 0), stop=(dt == NDT - 1))
                op = tmppool.tile([P, S], F32, name="outp", tag="tmp")
                nc.scalar.activation(op, ps, AF.Identity,
                                     bias=bp_sb[:, dto:dto + 1])
                nc.sync.dma_start(out_d[b, dto * P:(dto + 1) * P, :], op)

    if reps == 1:
        body()
    else:
        with tc.For_i(0, reps, 1) as i:
            body(i)
    ctx.close()


# ======================= host side =======================

def _prep_core_inputs(inputs):
    """Build the 8 per-core input maps (weights shared, x/tau/delta sharded)."""
    f = np.float32
    x = np.asarray(inputs["x"], f)
    tau = np.asarray(inputs["tau"], f)
    delta = np.asarray(inputs["delta"], f)
    scale = 1.0 / np.sqrt(np.float32(DH))

    shared = {}
    wq = np.asarray(inputs["Wq"], f)
    wk = np.asarray(inputs["Wk"], f)
    wv = np.asarray(inputs["Wv"], f)
    wo = np.asarray(inputs["Wo"], f)
    w1 = np.asarray(inputs["W1"], f)
    w2 = np.asarray(inputs["W2"], f)
    wp = np.asarray(inputs["Wp"], f)

    def col_tiled(wt):  # [L, D, D] (already [din, dout]) -> [L, NDT, D, P]
        return np.ascontiguousarray(
            wt.reshape(L, D, NDT, P).transpose(0, 2, 1, 3))

    wq_t = col_tiled(wq.transpose(0, 2, 1))
    wk_t = col_tiled(wk.transpose(0, 2, 1))
    wo_t = col_tiled(wo.transpose(0, 2, 1))
    wv_t = np.ascontiguousarray(wv.transpose(0, 2, 1))
    # W1 [L, F, D] -> W1^T [L, D, F] -> [L, NFC, NDT, P, FC]
    w1_tr = w1.transpose(0, 2, 1)
    w1_t = np.ascontiguousarray(
        w1_tr.reshape(L, NDT, P, NFC, FC).transpose(0, 3, 1, 2, 4))
    w2_t = np.ascontiguousarray(w2.transpose(0, 2, 1))  # [L, F, D]
    wp_t = np.ascontiguousarray(
        wp.transpose(1, 0).reshape(D, NDT, P).transpose(1, 0, 2))

    shared = {
        "wq_t": wq_t, "wk_t": wk_t, "wv_t": wv_t, "wo_t": wo_t,
        "w1_t": w1_t, "w2_t": w2_t, "wp_t": wp_t,
        "bq": np.asarray(inputs["bq"], f), "bk": np.asarray(inputs["bk"], f),
        "bv": np.asarray(inputs["bv"], f), "bo": np.asarray(inputs["bo"], f),
        "b1": np.asarray(inputs["b1"], f), "b2": np.asarray(inputs["b2"], f),
        "g1": np.asarray(inputs["g1"], f), "be1": np.asarray(inputs["be1"], f),
        "g2": np.asarray(inputs["g2"], f), "be2": np.asarray(inputs["be2"], f),
        "gf": np.asarray(inputs["gf"], f), "bf": np.asarray(inputs["bf"], f),
        "bp": np.asarray(inputs["bp"], f),
    }

    shared["ident"] = np.eye(P, dtype=f)

    in_maps = []
    for c in range(NCORES):
        bs = slice(c * BPC, (c + 1) * BPC)
        m = dict(shared)
        m["x_fm"] = np.ascontiguousarray(x[bs].transpose(0, 2, 1))
        m["sc_tau"] = np.tile(tau[bs] * scale, (1, P)).astype(f)
        m["sc_delta"] = (delta[bs] * scale).astype(f)
        in_maps.append(m)
    return in_maps


def run(inputs, reps=1):
    nc = _build(reps)
    in_maps = _prep_core_inputs(inputs)
    res = bass_utils.run_bass_kernel_spmd(nc, in_maps,
                                          core_ids=list(range(NCORES)))
    outs = [res.results[c]["out_fm"].transpose(0, 2, 1) for c in range(NCORES)]
    return np.ascontiguousarray(np.concatenate(outs, axis=0))


def kernel(**inputs) -> np.ndarray:
    return run(inputs, reps=1)



# revision 6
# speedup vs baseline: 2.4322x; 2.4322x over previous
"""Trainium2 Bass kernel for a 2-layer de-stationary-attention transformer.

Model (per reference):
  L=2 layers of: x += DSAttn(x); x = LN1(x); x = LN2(x + FFN(x)); then
  final LN + output projection Wp.
  DSAttn: softmax(scale * (Q K^T * tau + delta)) V with per-batch tau,
  per-(batch, key) delta.

Shapes: B=16, S=512, D=1024, H=16 heads (dh=64), F=4096.

Sharding: data-parallel over batch across 8 NeuronCores (2 batches/core),
weights replicated. No collectives.

v2 design notes:
  - All matmul operands bf16 (weights converted on host -> half the DMA
    bytes); fp32 PSUM accumulation; residual stream kept in f32r.
  - bf16 moving operands run at N=1024 (both batches per instruction),
    halving matmul instruction count vs fp32.
  - delta is folded into V: exp(scale*delta) scales V's columns (and
    replaces the ones-column that produces the softmax denominator), so
    exp(scores) needs only the per-batch tau scale -> one big ACT exp per
    score block instead of one per (key-tile).
  - LayerNorm: PE column-sum stats, rstd via Ln+Exp (stays in the exp
    table set), mean/rstd broadcast by K=1 matmuls then copied to SBUF so
    the per-tile normalize runs as two bf16 DVE ops at 2x rate.
  - FFN: all 32 h-tiles materialized in SBUF; y accumulated over the full
    F dimension in PSUM (two 4-d-tile waves x 8 banks); bias + residual
    fused into one scalar_tensor_tensor per output tile.
  - Residual adds fused with biases via scalar_tensor_tensor reading the
    matmul PSUM directly.
"""

import sys

if "/opt/trn_rl_repo" not in sys.path:
    sys.path.insert(0, "/opt/trn_rl_repo")

import numpy as np

import concourse.bass as bass
import concourse.bacc as bacc
import concourse.tile as tile
import concourse.mybir as mybir
from concourse import bass_utils

# Model dims
L, D, H, F = 2, 1024, 16, 4096
B, S = 16, 512
DH = D // H  # 64
NCORES = 8
BPC = B // NCORES   # batches per core
P = 128
NDT = D // P        # 8 d-tiles
NST = S // P        # 4 key-tiles per batch
NTOK = BPC * S      # 1024 tokens per core
NHP = H // 2        # 8 head pairs
NFT = F // P        # 32 f-tiles
VW = DH + 1         # 65: value width per head incl. denominator column
EPS = 1e-5

F32 = mybir.dt.float32
FR = mybir.dt.float32r
BF = mybir.dt.bfloat16
AF = mybir.ActivationFunctionType
ALU = mybir.AluOpType

_CACHE: dict = {}
import os
KGELU = os.environ.get("KGELU", "gelu")


def _build(reps: int):
    key = (reps, KGELU)
    if key in _CACHE:
        return _CACHE[key]

    nc = bacc.Bacc("TRN2", target_bir_lowering=False, debug=False,
                   num_devices=NCORES)

    # ---- DRAM tensors (per-core shapes) ----
    x_d = nc.dram_tensor("x_bf", (BPC, D, S), BF, kind="ExternalInput")
    wq_d = nc.dram_tensor("wq_t", (L, NHP, D, P), BF, kind="ExternalInput")
    wk_d = nc.dram_tensor("wk_t", (L, NHP, D, P), BF, kind="ExternalInput")
    wv_d = nc.dram_tensor("wv_t", (L, D, D), BF, kind="ExternalInput")
    wo_d = nc.dram_tensor("wo_t", (L, NDT, D, P), BF, kind="ExternalInput")
    w1_d = nc.dram_tensor("w1_t", (L, NFT, P, NDT, P), BF, kind="ExternalInput")
    w2_d = nc.dram_tensor("w2_t", (L, 2, NFT, P, 512), BF, kind="ExternalInput")
    wp_d = nc.dram_tensor("wp_t", (NDT, D, P), BF, kind="ExternalInput")
    bv_d = nc.dram_tensor("bv_bf", (L, D), BF, kind="ExternalInput")

    bq_d = nc.dram_tensor("bq", (L, D), F32, kind="ExternalInput")
    bk_d = nc.dram_tensor("bk", (L, D), F32, kind="ExternalInput")
    bo_d = nc.dram_tensor("bo", (L, D), F32, kind="ExternalInput")
    b1_d = nc.dram_tensor("b1", (L, F), F32, kind="ExternalInput")
    b2_d = nc.dram_tensor("b2", (L, D), F32, kind="ExternalInput")
    g1_d = nc.dram_tensor("g1", (L, D), F32, kind="ExternalInput")
    be1_d = nc.dram_tensor("be1", (L, D), F32, kind="ExternalInput")
    g2_d = nc.dram_tensor("g2", (L, D), F32, kind="ExternalInput")
    be2_d = nc.dram_tensor("be2", (L, D), F32, kind="ExternalInput")
    gf_d = nc.dram_tensor("gf", (D,), F32, kind="ExternalInput")
    bf_d = nc.dram_tensor("bf", (D,), F32, kind="ExternalInput")
    bp_d = nc.dram_tensor("bp", (D,), F32, kind="ExternalInput")
    stau_d = nc.dram_tensor("sc_tau", (BPC, P), F32, kind="ExternalInput")
    edc_d = nc.dram_tensor("ed_cols", (P, NDT), F32, kind="ExternalInput")
    edr_d = nc.dram_tensor("ed_rep", (P, NDT, H), F32, kind="ExternalInput")

    out_d = nc.dram_tensor("out_fm", (BPC, D, S), F32, kind="ExternalOutput")

    with tile.TileContext(nc) as tc:
        _emit(nc, tc, reps, locals())

    nc.compile()
    _CACHE[key] = nc
    return nc


def _emit(nc, tc, reps, d):
    x_d, wq_d, wk_d, wv_d, wo_d, w1_d, w2_d, wp_d = (
        d["x_d"], d["wq_d"], d["wk_d"], d["wv_d"], d["wo_d"], d["w1_d"],
        d["w2_d"], d["wp_d"])
    bv_d, bq_d, bk_d, bo_d, b1_d, b2_d = (
        d["bv_d"], d["bq_d"], d["bk_d"], d["bo_d"], d["b1_d"], d["b2_d"])
    g1_d, be1_d, g2_d, be2_d, gf_d, bf_d, bp_d = (
        d["g1_d"], d["be1_d"], d["g2_d"], d["be2_d"], d["gf_d"], d["bf_d"],
        d["bp_d"])
    stau_d, edc_d, edr_d, out_d = (
        d["stau_d"], d["edc_d"], d["edr_d"], d["out_d"])

    from contextlib import ExitStack
    ctx = ExitStack()
    singles = ctx.enter_context(tc.tile_pool(name="singles", bufs=1))
    xpool = ctx.enter_context(tc.tile_pool(name="xpool", bufs=1))
    xbpool = ctx.enter_context(tc.tile_pool(name="xbpool", bufs=1))
    vhpool = ctx.enter_context(tc.tile_pool(name="vhpool", bufs=1))
    qkpool = ctx.enter_context(tc.tile_pool(name="qkpool", bufs=4))
    etpool = ctx.enter_context(tc.tile_pool(name="etpool", bufs=2))
    wpool = ctx.enter_context(tc.tile_pool(name="wpool", bufs=8))
    tmppool = ctx.enter_context(tc.tile_pool(name="tmppool", bufs=4))
    outpool = ctx.enter_context(tc.tile_pool(name="outpool", bufs=1))
    rowpool = ctx.enter_context(tc.tile_pool(name="rowpool", bufs=4))
    psum = ctx.enter_context(tc.tile_pool(name="psum", bufs=4, space="PSUM"))

    # ---- constants / params (loaded once, outside the reps loop) ----
    ones_f = singles.tile([P, 1], F32)
    nc.vector.memset(ones_f, 1.0)
    ones_col_fr = singles.tile([P, 1], FR)
    nc.scalar.activation(ones_col_fr, ones_f, AF.Copy)
    ones_col_bf = singles.tile([P, 1], BF)
    nc.scalar.activation(ones_col_bf, ones_f, AF.Copy)
    ones_rowf = singles.tile([1, P], F32)
    nc.vector.memset(ones_rowf, 1.0)
    ones_row_fr = singles.tile([1, P], FR)
    nc.scalar.activation(ones_row_fr, ones_rowf, AF.Copy)
    ones_row_bf = singles.tile([1, P], BF)
    nc.scalar.activation(ones_row_bf, ones_rowf, AF.Copy)
    eps_row = singles.tile([1, 1], F32)
    nc.vector.memset(eps_row, EPS)

    def load_cols(dram_row, ncols):
        t = singles.tile([P, ncols], dram_row.dtype,
                         name=f"prm_{dram_row.tensor.name}_{nc.next_id()}")
        nc.sync.dma_start(t, dram_row.rearrange("(t p) -> p t", p=P))
        return t

    bq_sb, bk_sb, bo_sb, b2_sb = [], [], [], []
    g1_sb, be1_sb, g2_sb, be2_sb, b1_sb = [], [], [], [], []
    bv_sb = singles.tile([1, L * D], BF)
    for l in range(L):
        bq_sb.append(load_cols(bq_d[l], NHP))
        bk_sb.append(load_cols(bk_d[l], NHP))
        bo_sb.append(load_cols(bo_d[l], NDT))
        b2_sb.append(load_cols(b2_d[l], NDT))
        g1_sb.append(load_cols(g1_d[l], NDT))
        be1_sb.append(load_cols(be1_d[l], NDT))
        g2_sb.append(load_cols(g2_d[l], NDT))
        be2_sb.append(load_cols(be2_d[l], NDT))
        b1_sb.append(load_cols(b1_d[l], NFT))
        nc.sync.dma_start(bv_sb[:, l * D:(l + 1) * D], bv_d[l][None, :])
    gf_sb = load_cols(gf_d.ap(), NDT)
    bf_sb = load_cols(bf_d.ap(), NDT)
    bp_sb = load_cols(bp_d.ap(), NDT)
    stau_sb = singles.tile([P, BPC], F32)
    nc.sync.dma_start(stau_sb, stau_d.ap().rearrange("b p -> p b"))
    edc_sb = singles.tile([P, NDT], F32)
    nc.sync.dma_start(edc_sb, edc_d.ap())
    edr_sb = singles.tile([P, NDT * H], F32)
    nc.sync.dma_start(edr_sb.rearrange("p (t h) -> p t h", t=NDT), edr_d.ap())

    gelu_f = AF.Gelu if KGELU == "gelu" else AF.Identity

    def body(_i=None):
        # ---- load x (feature-major, bf16) ----
        x_sb = []   # residual stream, f32r
        xb_sb = []  # bf16 matmul-operand view of the stream
        for dt in range(NDT):
            xt = xpool.tile([P, NTOK], FR, name=f"x_{dt}", tag=f"x_{dt}")
            x_sb.append(xt)
            xbt = xbpool.tile([P, NTOK], BF, name=f"xb_{dt}", tag=f"xb_{dt}")
            for b in range(BPC):
                nc.sync.dma_start(xbt[:, b * S:(b + 1) * S],
                                  x_d[b, dt * P:(dt + 1) * P, :])
            xb_sb.append(xbt)

        def ln(src, g_t, be_t, first=False):
            """LayerNorm over d (partitions): src = 8 f32r tiles [P, NTOK];
            writes normalized bf16 into xb_sb. Stats for both batches land
            in one PSUM row [1, NTOK]; rstd = exp(-0.5*ln(var+eps)) keeps
            ACT in the exp/ln table set."""
            ps_s = psum.tile([P, NTOK], F32, name="ps_s", tag="ps")
            for b in range(BPC):
                cs = slice(b * S, (b + 1) * S)
                for dt in range(NDT):
                    nc.tensor.matmul(ps_s[0:1, cs], ones_col_fr,
                                     src[dt][:, cs],
                                     start=(dt == 0), stop=(dt == NDT - 1))
            ps_q = psum.tile([P, NTOK], F32, name="ps_q", tag="ps")
            for dt in range(NDT):
                sq = tmppool.tile([P, NTOK], BF, name="sq", tag="tmp")
                nc.scalar.activation(sq, src[dt], AF.Square)
                nc.tensor.matmul(ps_q[0:1, :], ones_col_bf, sq,
                                 start=(dt == 0), stop=(dt == NDT - 1))
            mean_n = rowpool.tile([1, NTOK], FR, name="mean_n", tag="row")
            nc.vector.tensor_scalar(mean_n, ps_s[0:1, :], -1.0 / D, None,
                                    ALU.mult)
            m2 = rowpool.tile([1, NTOK], F32, name="m2", tag="row")
            nc.vector.tensor_mul(m2, mean_n, mean_n)
            var = rowpool.tile([1, NTOK], F32, name="var", tag="row")
            nc.vector.scalar_tensor_tensor(var, ps_q[0:1, :], 1.0 / D, m2,
                                           ALU.mult, ALU.subtract)
            lnv = rowpool.tile([1, NTOK], F32, name="lnv", tag="row")
            nc.scalar.activation(lnv, var, AF.Ln, bias=eps_row)
            rstd = rowpool.tile([1, NTOK], FR, name="rstd", tag="row")
            nc.scalar.activation(rstd, lnv, AF.Exp, scale=-0.5)
            # broadcast -mean and rstd across partitions, then to SBUF bf16
            pm = psum.tile([P, NTOK], F32, name="pm", tag="ps")
            pr = psum.tile([P, NTOK], F32, name="pr", tag="ps")
            for b in range(BPC):
                cs = slice(b * S, (b + 1) * S)
                nc.tensor.matmul(pm[:, cs], ones_row_fr, mean_n[:, cs])
                nc.tensor.matmul(pr[:, cs], ones_row_fr, rstd[:, cs])
            mb = tmppool.tile([P, NTOK], BF, name="mb", tag="tmp")
            nc.scalar.activation(mb, pm, AF.Copy)
            rb = tmppool.tile([P, NTOK], BF, name="rb", tag="tmp")
            nc.scalar.activation(rb, pr, AF.Copy)
            for dt in range(NDT):
                t1 = tmppool.tile([P, NTOK], BF, name="t1", tag="tmp")
                nc.vector.tensor_add(t1, src[dt], mb)
                t2 = tmppool.tile([P, NTOK], BF, name="t2", tag="tmp")
                nc.vector.tensor_mul(t2, t1, rb)
                nc.scalar.activation(xb_sb[dt], t2, AF.Identity,
                                     scale=g_t[:, dt:dt + 1],
                                     bias=be_t[:, dt:dt + 1])

        def attn_phase(l):
            # ---- V (token-major; denominator column = exp(delta)) ----
            wv_sb = []
            for dt in range(NDT):
                wt = wpool.tile([P, D], BF, name=f"wv_{dt}", tag="w")
                nc.sync.dma_start(wt, wv_d[l, dt * P:(dt + 1) * P, :])
                wv_sb.append(wt)
            v_sb = []
            for tt in range(NDT):
                vt = vhpool.tile([P, H * VW], BF, name=f"v_{tt}",
                                 tag=f"vh_{tt}")
                nc.scalar.activation(
                    vt.rearrange("p (h e) -> p h e", e=VW)[:, :, DH:DH + 1],
                    edr_sb.rearrange("p (t h) -> p t h", t=NDT)
                    [:, tt:tt + 1, :].rearrange("p t h -> p h t"),
                    AF.Copy)
                v_sb.append(vt)
            for tt in range(NDT):
                ts = slice(tt * P, (tt + 1) * P)
                ps = psum.tile([P, NTOK], F32, name="ps_v", tag="ps")
                for dt in range(NDT):
                    nc.tensor.matmul(ps, xb_sb[dt][:, ts], wv_sb[dt],
                                     start=(dt == 0), stop=False)
                nc.tensor.matmul(ps, ones_row_bf[:, :P],
                                 bv_sb[:, l * D:(l + 1) * D],
                                 start=False, stop=True)
                nc.scalar.activation(
                    v_sb[tt].rearrange("p (h e) -> p h e", e=VW)[:, :, 0:DH],
                    ps.rearrange("p (h e) -> p h e", e=DH),
                    AF.Identity, scale=edc_sb[:, tt:tt + 1])

            # ---- per head pair: Q, K, scores, exp, AV, normalize ----
            o_sb = []
            for hp in range(NHP):
                ot = opool.tile([P, NTOK], BF, name=f"o_{hp}", tag=f"o_{hp}")
                o_sb.append(ot)
            pending = []

            def flush():
                if pending:
                    pending.pop(0)()

            for hp in range(NHP):
                wq_p = wpool.tile([P, NDT, P], BF, name="wq_p", tag="w")
                nc.sync.dma_start(
                    wq_p, wq_d[l, hp].rearrange("(t p) m -> p t m", p=P))
                wk_p = wpool.tile([P, NDT, P], BF, name="wk_p", tag="w")
                nc.sync.dma_start(
                    wk_p, wk_d[l, hp].rearrange("(t p) m -> p t m", p=P))
                q_p = qkpool.tile([P, NTOK], BF, name="q_p", tag="qk")
                k_p = qkpool.tile([P, NTOK], BF, name="k_p", tag="qk")
                for wt, dst, bias in ((wq_p, q_p, bq_sb[l]),
                                      (wk_p, k_p, bk_sb[l])):
                    ps = psum.tile([P, NTOK], F32, name="ps_qk", tag="ps")
                    for dt in range(NDT):
                        nc.tensor.matmul(ps, wt[:, dt, :], xb_sb[dt],
                                         start=(dt == 0),
                                         stop=(dt == NDT - 1))
                    nc.scalar.activation(dst, ps, AF.Identity,
                                         bias=bias[:, hp:hp + 1])
                for b in range(BPC):
                    cs = slice(b * S, (b + 1) * S)
                    for lh in range(2):
                        h = hp * 2 + lh
                        rsl = slice(lh * DH, (lh + 1) * DH)
                        et = etpool.tile([P, 2 * NTOK], BF, name="et",
                                         tag="et")
                        for half in range(2):
                            ps = psum.tile([P, NTOK], F32, name="ps_sc",
                                           tag="ps")
                            for j in range(2):
                                st = half * 2 + j
                                nc.tensor.matmul(
                                    ps[:, j * S:(j + 1) * S],
                                    k_p[rsl,
                                        b * S + st * P: b * S + (st + 1) * P],
                                    q_p[rsl, cs])
                            nc.scalar.activation(
                                et[:, half * NTOK:(half + 1) * NTOK], ps,
                                AF.Exp, scale=stau_sb[:, b:b + 1])

                        def s2(et=et, ot=o_sb[hp], cs=cs, h=h, b=b, rsl=rsl):
                            pav = psum.tile([P, NTOK], F32, name="pav",
                                            tag="ps")
                            for st in range(NST):
                                nc.tensor.matmul(
                                    pav[0:VW, 0:S],
                                    v_sb[b * NST + st][:, h * VW:(h + 1) * VW],
                                    et[:, st * S:(st + 1) * S],
                                    start=(st == 0), stop=(st == NST - 1))
                            den_r = rowpool.tile([1, S], FR, name="den_r",
                                                 tag="row")
                            with nc.allow_low_precision(
                                    reason="f32r rows feed matmuls"):
                                nc.vector.reciprocal(den_r, pav[DH:DH + 1, 0:S])
                            # broadcast lands on rows 64:128 (32-aligned
                            # base); row 64 (the den row) is overwritten
                            # after the reciprocal has read it.
                            nc.tensor.matmul(pav[DH:2 * DH, 0:S],
                                             ones_row_fr[:, :DH], den_r)
                            nc.vector.tensor_mul(ot[rsl, cs], pav[0:DH, 0:S],
                                                 pav[DH:2 * DH, 0:S])

                        pending.append(s2)
                        if len(pending) > 1:
                            flush()
            while pending:
                flush()

            # ---- Wo projection + bias + residual (into x_sb f32r) ----
            for dto in range(NDT):
                wo_p = wpool.tile([P, NDT, P], BF, name="wo_p", tag="w")
                nc.sync.dma_start(
                    wo_p, wo_d[l, dto].rearrange("(t p) m -> p t m", p=P))
                ps = psum.tile([P, NTOK], F32, name="ps_wo", tag="ps")
                for dt in range(NDT):
                    nc.tensor.matmul(ps, wo_p[:, dt, :], o_sb[dt],
                                     start=(dt == 0), stop=(dt == NDT - 1))
                with nc.allow_low_precision(reason="residual in f32r"):
                    nc.vector.scalar_tensor_tensor(
                        x_sb[dto], ps, bo_sb[l][:, dto:dto + 1], xb_sb[dto],
                        ALU.add, ALU.add)

        def ffn_phase(l):
            # ---- h = gelu(W1 z + b1), all 32 f-tiles resident ----
            h_sb = []
            for ft in range(NFT):
                w1_p = wpool.tile([P, NDT, P], BF, name="w1_p", tag="w")
                nc.sync.dma_start(w1_p, w1_d[l, ft])
                ps = psum.tile([P, NTOK], F32, name="ps_h", tag="ps")
                for dt in range(NDT):
                    nc.tensor.matmul(ps, w1_p[:, dt, :], xb_sb[dt],
                                     start=(dt == 0), stop=(dt == NDT - 1))
                ht = vhpool.tile([P, NTOK], BF, name="htile", tag=f"vh_{ft}")
                nc.scalar.activation(ht, ps, gelu_f,
                                     bias=b1_sb[l][:, ft:ft + 1])
                h_sb.append(ht)
            # ---- y = W2 h (full-F PSUM accumulation, 2 waves of 4 dto) ----
            for half in range(2):
                ys = []
                for j in range(4):
                    yp = psum.tile([P, NTOK], F32, name="ps_y", tag="ps")
                    ys.append(yp)
                for ft in range(NFT):
                    w2_p = wpool.tile([P, 512], BF, name="w2_p", tag="w")
                    nc.sync.dma_start(w2_p, w2_d[l, half, ft])
                    for j in range(4):
                        nc.tensor.matmul(ys[j], w2_p[:, j * P:(j + 1) * P],
                                         h_sb[ft],
                                         start=(ft == 0), stop=(ft == NFT - 1))
                for j in range(4):
                    dto = half * 4 + j
                    with nc.allow_low_precision(reason="residual in f32r"):
                        nc.vector.scalar_tensor_tensor(
                            x_sb[dto], ys[j], b2_sb[l][:, dto:dto + 1],
                            xb_sb[dto], ALU.add, ALU.add)

        for l in range(L):
            attn_phase(l)
            ln(x_sb, g1_sb[l], be1_sb[l])
            ffn_phase(l)
            ln(x_sb, g2_sb[l], be2_sb[l])

        # ---- final LN + Wp ----
        ln(x_sb, gf_sb, bf_sb)
        for dto in range(NDT):
            wp_p = wpool.tile([P, NDT, P], BF, name="wp_p", tag="w")
            nc.sync.dma_start(wp_p,
                              wp_d[dto].rearrange("(t p) m -> p t m", p=P))
            ps = psum.tile([P, NTOK], F32, name="ps_wp", tag="ps")
            for dt in range(NDT):
                nc.tensor.matmul(ps, wp_p[:, dt, :], xb_sb[dt],
                                 start=(dt == 0), stop=(dt == NDT - 1))
            op = outpool.tile([P, NTOK], F32, name="outp", tag="out")
            nc.scalar.activation(op, ps, AF.Identity,
                                 bias=bp_sb[:, dto:dto + 1])
            for b in range(BPC):
                nc.sync.dma_start(out_d[b, dto * P:(dto + 1) * P, :],
                                  op[:, b * S:(b + 1) * S])

    if reps == 1:
        body()
    else:
        with tc.For_i(0, reps, 1) as i:
            body(i)
    ctx.close()


# ======================= host side =======================

def _prep_core_inputs(inputs):
    """Build the 8 per-core input maps (weights shared, x/tau/delta sharded)."""
    import ml_dtypes
    bf = ml_dtypes.bfloat16
    f = np.float32
    x = np.asarray(inputs["x"], f)
    tau = np.asarray(inputs["tau"], f)
    delta = np.asarray(inputs["delta"], f)
    scale = 1.0 / np.sqrt(np.float32(DH))

    wq = np.asarray(inputs["Wq"], f)
    wk = np.asarray(inputs["Wk"], f)
    wv = np.asarray(inputs["Wv"], f)
    wo = np.asarray(inputs["Wo"], f)
    w1 = np.asarray(inputs["W1"], f)
    w2 = np.asarray(inputs["W2"], f)
    wp = np.asarray(inputs["Wp"], f)

    def hp_tiled(wt):  # [L, din, dout] -> [L, NHP, din, P]
        return np.ascontiguousarray(
            wt.reshape(L, D, NHP, P).transpose(0, 2, 1, 3)).astype(bf)

    wq_t = hp_tiled(wq.transpose(0, 2, 1))
    wk_t = hp_tiled(wk.transpose(0, 2, 1))
    wo_t = hp_tiled(wo.transpose(0, 2, 1))
    wv_t = np.ascontiguousarray(wv.transpose(0, 2, 1)).astype(bf)
    # W1 [L, F, D] -> W1^T [L, D, F] -> [L, NFT, P(d), NDT, P(f)]
    w1_t = np.ascontiguousarray(
        w1.transpose(0, 2, 1).reshape(L, NDT, P, NFT, P)
        .transpose(0, 3, 2, 1, 4)).astype(bf)
    # W2 [L, D, F] -> W2^T [L, F, D] -> [L, 2, NFT, P(f), 512(dto cols)]
    w2_t = np.ascontiguousarray(
        w2.transpose(0, 2, 1).reshape(L, NFT, P, 2, 512)
        .transpose(0, 3, 1, 2, 4)).astype(bf)
    wp_t = np.ascontiguousarray(
        wp.transpose(1, 0).reshape(D, NDT, P).transpose(1, 0, 2)).astype(bf)

    shared = {
        "wq_t": wq_t, "wk_t": wk_t, "wv_t": wv_t, "wo_t": wo_t,
        "w1_t": w1_t, "w2_t": w2_t, "wp_t": wp_t,
        "bv_bf": np.asarray(inputs["bv"], f).astype(bf),
        "bq": np.asarray(inputs["bq"], f), "bk": np.asarray(inputs["bk"], f),
        "bo": np.asarray(inputs["bo"], f),
        "b1": np.asarray(inputs["b1"], f), "b2": np.asarray(inputs["b2"], f),
        "g1": np.asarray(inputs["g1"], f), "be1": np.asarray(inputs["be1"], f),
        "g2": np.asarray(inputs["g2"], f), "be2": np.asarray(inputs["be2"], f),
        "gf": np.asarray(inputs["gf"], f), "bf": np.asarray(inputs["bf"], f),
        "bp": np.asarray(inputs["bp"], f),
    }

    in_maps = []
    for c in range(NCORES):
        bs = slice(c * BPC, (c + 1) * BPC)
        m = dict(shared)
        m["x_bf"] = np.ascontiguousarray(
            x[bs].transpose(0, 2, 1)).astype(bf)
        m["sc_tau"] = np.tile(tau[bs] * scale, (1, P)).astype(f)
        ed = np.exp(delta[bs] * scale).astype(f)          # [BPC, S]
        edc = np.ascontiguousarray(
            ed.reshape(BPC, NST, P).transpose(2, 0, 1).reshape(P, NDT))
        m["ed_cols"] = edc
        m["ed_rep"] = np.ascontiguousarray(
            np.repeat(edc[:, :, None], H, axis=2))
        in_maps.append(m)
    return in_maps


def run(inputs, reps=1):
    nc = _build(reps)
    in_maps = _prep_core_inputs(inputs)
    res = bass_utils.run_bass_kernel_spmd(nc, in_maps,
                                          core_ids=list(range(NCORES)))
    outs = [res.results[c]["out_fm"].transpose(0, 2, 1) for c in range(NCORES)]
    return np.ascontiguousarray(np.concatenate(outs, axis=0))


def kernel(**inputs) -> np.ndarray:
    return run(inputs, reps=1)


# revision 16
# speedup vs baseline: 3.1336x; 1.2884x over previous
"""Trainium2 Bass kernel for a 2-layer de-stationary-attention transformer.

Model (per reference):
  L=2 layers of: x += DSAttn(x); x = LN1(x); x = LN2(x + FFN(x)); then
  final LN + output projection Wp.
  DSAttn: softmax(scale * (Q K^T * tau + delta)) V with per-batch tau,
  per-(batch, key) delta.

Shapes: B=16, S=512, D=1024, H=16 heads (dh=64), F=4096.

Sharding: data-parallel over batch across 8 NeuronCores (2 batches/core),
weights replicated. No collectives.

v2 design notes:
  - All matmul operands bf16 (weights converted on host -> half the DMA
    bytes); fp32 PSUM accumulation; residual stream kept in f32r.
  - bf16 moving operands run at N=1024 (both batches per instruction),
    halving matmul instruction count vs fp32.
  - delta is folded into V: exp(scale*delta) scales V's columns (and
    replaces the ones-column that produces the softmax denominator), so
    exp(scores) needs only the per-batch tau scale -> one big ACT exp per
    score block instead of one per (key-tile).
  - LayerNorm: PE column-sum stats, rstd via Ln+Exp (stays in the exp
    table set), mean/rstd broadcast by K=1 matmuls then copied to SBUF so
    the per-tile normalize runs as two bf16 DVE ops at 2x rate.
  - FFN: all 32 h-tiles materialized in SBUF; y accumulated over the full
    F dimension in PSUM (two 4-d-tile waves x 8 banks); bias + residual
    fused into one scalar_tensor_tensor per output tile.
  - Residual adds fused with biases via scalar_tensor_tensor reading the
    matmul PSUM directly.
"""

import sys

if "/opt/trn_rl_repo" not in sys.path:
    sys.path.insert(0, "/opt/trn_rl_repo")

import numpy as np

import concourse.bass as bass
import concourse.bacc as bacc
import concourse.tile as tile
import concourse.mybir as mybir
from concourse import bass_utils

# Model dims
L, D, H, F = 2, 1024, 16, 4096
B, S = 16, 512
DH = D // H  # 64
NCORES = 8
BPC = B // NCORES   # batches per core
P = 128
NDT = D // P        # 8 d-tiles
NST = S // P        # 4 key-tiles per batch
NTOK = BPC * S      # 1024 tokens per core
NHP = H // 2        # 8 head pairs
NFT = F // P        # 32 f-tiles
VW = DH + 1         # 65: value width per head incl. denominator column
EPS = 1e-5

F32 = mybir.dt.float32
FR = mybir.dt.float32r
BF = mybir.dt.bfloat16
AF = mybir.ActivationFunctionType
ALU = mybir.AluOpType

_CACHE: dict = {}
import os
KGELU = os.environ.get("KGELU", "gelu")


def _build(reps: int):
    key = (reps, KGELU)
    if key in _CACHE:
        return _CACHE[key]

    nc = bacc.Bacc("TRN2", target_bir_lowering=False, debug=False,
                   num_devices=NCORES)

    # ---- DRAM tensors (per-core shapes) ----
    x_d = nc.dram_tensor("x_bf", (BPC, D, S), BF, kind="ExternalInput")
    wq_d = nc.dram_tensor("wq_t", (L, NHP, D, P), BF, kind="ExternalInput")
    wk_d = nc.dram_tensor("wk_t", (L, NHP, D, P), BF, kind="ExternalInput")
    wv_d = nc.dram_tensor("wv_t", (L, D, D), BF, kind="ExternalInput")
    wo_d = nc.dram_tensor("wo_t", (L, NDT, D, P), BF, kind="ExternalInput")
    w1_d = nc.dram_tensor("w1_t", (L, NFT, P, NDT, P), BF, kind="ExternalInput")
    w2_d = nc.dram_tensor("w2_t", (L, 2, NFT, P, 512), BF, kind="ExternalInput")
    wp_d = nc.dram_tensor("wp_t", (NDT, D, P), BF, kind="ExternalInput")
    bv_d = nc.dram_tensor("bv_bf", (L, D), BF, kind="ExternalInput")

    # all [P, 1]-sliceable f32 params packed into one column array:
    # per l: bq 8 | bk 8 | bo 8 | b2 8 | g1 8 | be1 8 | g2 8 | be2 8 | b1 32
    # then gf 8 | bf 8 | bp 8 | stau 2 | edc 8 | edr 128
    NPC = 96 * L + 8 * 3 + BPC + NDT + NDT * H
    pc_d = nc.dram_tensor("pcols", (P, NPC), F32, kind="ExternalInput")

    out_d = nc.dram_tensor("out_fm", (BPC, D, S), F32, kind="ExternalOutput")

    with tile.TileContext(nc) as tc:
        _emit(nc, tc, reps, locals())

    nc.compile()
    _CACHE[key] = nc
    return nc


def _emit(nc, tc, reps, d):
    x_d, wq_d, wk_d, wv_d, wo_d, w1_d, w2_d, wp_d = (
        d["x_d"], d["wq_d"], d["wk_d"], d["wv_d"], d["wo_d"], d["w1_d"],
        d["w2_d"], d["wp_d"])
    bv_d, pc_d, out_d, NPC = d["bv_d"], d["pc_d"], d["out_d"], d["NPC"]

    from contextlib import ExitStack
    ctx = ExitStack()
    singles = ctx.enter_context(tc.tile_pool(name="singles", bufs=1))
    xpool = ctx.enter_context(tc.tile_pool(name="xpool", bufs=1))
    xbpool = ctx.enter_context(tc.tile_pool(name="xbpool", bufs=1))
    vhpool = ctx.enter_context(tc.tile_pool(name="vhpool", bufs=1))
    qkpool = ctx.enter_context(tc.tile_pool(name="qkpool", bufs=4))
    etpool = ctx.enter_context(tc.tile_pool(name="etpool", bufs=2))
    wpool = ctx.enter_context(tc.tile_pool(name="wpool", bufs=8))
    tmppool = ctx.enter_context(tc.tile_pool(name="tmppool", bufs=4))
    outpool = ctx.enter_context(tc.tile_pool(name="outpool", bufs=1))
    rowpool = ctx.enter_context(tc.tile_pool(name="rowpool", bufs=4))
    psum = ctx.enter_context(tc.tile_pool(name="psum", bufs=4, space="PSUM"))

    # ---- constants / params (loaded once, outside the reps loop) ----
    ones_f = singles.tile([P, 1], F32)
    nc.vector.memset(ones_f, 1.0)
    ones_col_fr = singles.tile([P, 1], FR)
    nc.scalar.activation(ones_col_fr, ones_f, AF.Copy)
    ones_col_bf = singles.tile([P, 1], BF)
    nc.scalar.activation(ones_col_bf, ones_f, AF.Copy)
    ones_rowf = singles.tile([1, P], F32)
    nc.vector.memset(ones_rowf, 1.0)
    ones_row_fr = singles.tile([1, P], FR)
    nc.scalar.activation(ones_row_fr, ones_rowf, AF.Copy)
    ones_row_bf = singles.tile([1, P], BF)
    nc.scalar.activation(ones_row_bf, ones_rowf, AF.Copy)
    eps_row = singles.tile([1, 1], F32)
    nc.vector.memset(eps_row, EPS)

    pc_sb = singles.tile([P, NPC], F32)
    nc.sync.dma_start(pc_sb, pc_d.ap())
    bv_sb = singles.tile([1, L * D], BF)
    nc.sync.dma_start(bv_sb, bv_d.ap().rearrange("l d -> (l d)")[None, :])

    _off = [0]

    def cols(n):
        c = pc_sb[:, _off[0]:_off[0] + n]
        _off[0] += n
        return c

    bq_sb, bk_sb, bo_sb, b2_sb = [], [], [], []
    g1_sb, be1_sb, g2_sb, be2_sb, b1_sb = [], [], [], [], []
    for l in range(L):
        bq_sb.append(cols(NHP))
        bk_sb.append(cols(NHP))
        bo_sb.append(cols(NDT))
        b2_sb.append(cols(NDT))
        g1_sb.append(cols(NDT))
        be1_sb.append(cols(NDT))
        g2_sb.append(cols(NDT))
        be2_sb.append(cols(NDT))
        b1_sb.append(cols(NFT))
    gf_sb = cols(NDT)
    bf_sb = cols(NDT)
    bp_sb = cols(NDT)
    stau_sb = cols(BPC)
    edc_sb = cols(NDT)
    edr_sb = cols(NDT * H)

    gelu_f = AF.Gelu if KGELU == "gelu" else AF.Identity

    def mm2(out, lhsT, rhs, start, stop):
        """Matmul with N=1024 moving operand split into two N=512 halves
        (matmul output must stay within one PSUM bank)."""
        for h2 in range(2):
            fs = slice(h2 * S, (h2 + 1) * S)
            nc.tensor.matmul(out[:, fs], lhsT, rhs[:, fs],
                             start=start, stop=stop)

    def body(_i=None):
        # ---- load x (feature-major, bf16) ----
        x_sb = []   # residual stream, f32r
        xb_sb = []  # bf16 matmul-operand view of the stream
        for dt in range(NDT):
            xt = xpool.tile([P, NTOK], FR, name=f"x_{dt}", tag=f"x_{dt}")
            x_sb.append(xt)
            xbt = xbpool.tile([P, NTOK], BF, name=f"xb_{dt}", tag=f"xb_{dt}")
            nc.sync.dma_start(
                xbt.rearrange("p (b s) -> p b s", b=BPC),
                x_d.ap()[:, dt * P:(dt + 1) * P, :].rearrange(
                    "b p s -> p b s"))
            xb_sb.append(xbt)

        def ln(src, g_t, be_t):
            """LayerNorm over d (partitions): src = 8 tiles [P, NTOK] (f32r
            residual or bf16 xb); writes normalized bf16 into xb_sb. Stats
            for both batches land in one PSUM row [1, NTOK];
            rstd = exp(-0.5*ln(var+eps)) keeps ACT in the exp table set."""
            src_bf = src[0].dtype == BF
            ps_s = psum.tile([P, NTOK], F32, name="ps_s", tag="ps")
            if src_bf:
                for dt in range(NDT):
                    mm2(ps_s[0:1, :], ones_col_bf, src[dt],
                        start=(dt == 0), stop=(dt == NDT - 1))
            else:
                for b in range(BPC):
                    cs = slice(b * S, (b + 1) * S)
                    for dt in range(NDT):
                        nc.tensor.matmul(ps_s[0:1, cs], ones_col_fr,
                                         src[dt][:, cs],
                                         start=(dt == 0),
                                         stop=(dt == NDT - 1))
            ps_q = psum.tile([P, NTOK], F32, name="ps_q", tag="ps")
            for dt in range(NDT):
                sq = tmppool.tile([P, NTOK], BF, name="sq", tag="tmp")
                nc.scalar.activation(sq, src[dt], AF.Square)
                mm2(ps_q[0:1, :], ones_col_bf, sq,
                    start=(dt == 0), stop=(dt == NDT - 1))
            mean_n = rowpool.tile([1, NTOK], FR, name="mean_n", tag="row")
            nc.vector.tensor_scalar(mean_n, ps_s[0:1, :], -1.0 / D, None,
                                    ALU.mult)
            m2 = rowpool.tile([1, NTOK], F32, name="m2", tag="row")
            nc.vector.tensor_mul(m2, mean_n, mean_n)
            var = rowpool.tile([1, NTOK], F32, name="var", tag="row")
            nc.vector.scalar_tensor_tensor(var, ps_q[0:1, :], 1.0 / D, m2,
                                           ALU.mult, ALU.subtract)
            lnv = rowpool.tile([1, NTOK], F32, name="lnv", tag="row")
            nc.scalar.activation(lnv, var, AF.Ln, bias=eps_row)
            rstd = rowpool.tile([1, NTOK], FR, name="rstd", tag="row")
            nc.scalar.activation(rstd, lnv, AF.Exp, scale=-0.5)
            # broadcast -mean and rstd across partitions, then to SBUF bf16
            pm = psum.tile([P, NTOK], F32, name="pm", tag="ps")
            pr = psum.tile([P, NTOK], F32, name="pr", tag="ps")
            for b in range(BPC):
                cs = slice(b * S, (b + 1) * S)
                nc.tensor.matmul(pm[:, cs], ones_row_fr, mean_n[:, cs])
                nc.tensor.matmul(pr[:, cs], ones_row_fr, rstd[:, cs])
            mb = tmppool.tile([P, NTOK], BF, name="mb", tag="mb", bufs=1)
            nc.scalar.activation(mb, pm, AF.Copy)
            rb = tmppool.tile([P, NTOK], BF, name="rb", tag="rb", bufs=1)
            nc.scalar.activation(rb, pr, AF.Copy)
            for dt in range(NDT):
                t1 = tmppool.tile([P, NTOK], BF, name="t1", tag="tmp")
                nc.vector.tensor_add(t1, src[dt], mb)
                t2 = tmppool.tile([P, NTOK], BF, name="t2", tag="tmp")
                nc.vector.tensor_mul(t2, t1, rb)
                nc.scalar.activation(xb_sb[dt], t2, AF.Identity,
                                     scale=g_t[:, dt:dt + 1],
                                     bias=be_t[:, dt:dt + 1])

        def attn_phase(l):
            # ---- V (token-major; denominator column = exp(delta)) ----
            wv_sb = []
            for dt in range(NDT):
                wt = wpool.tile([P, D], BF, name=f"wv_{dt}", tag="w")
                nc.sync.dma_start(wt, wv_d[l, dt * P:(dt + 1) * P, :])
                wv_sb.append(wt)
            v_sb = []
            for tt in range(NDT):
                vt = vhpool.tile([P, H * VW], BF, name=f"v_{tt}",
                                 tag=f"vh_{tt}")
                nc.scalar.activation(
                    vt.rearrange("p (h e) -> p h e", e=VW)[:, :, DH:DH + 1],
                    edr_sb[:, tt * H:(tt + 1) * H]
                    .rearrange("p (h o) -> p h o", o=1),
                    AF.Copy)
                v_sb.append(vt)
            for tt in range(NDT):
                ts = slice(tt * P, (tt + 1) * P)
                ps = psum.tile([P, NTOK], F32, name="ps_v", tag="ps")
                for dt in range(NDT):
                    mm2(ps, xb_sb[dt][:, ts], wv_sb[dt],
                        start=(dt == 0), stop=False)
                mm2(ps, ones_row_bf[:, :P], bv_sb[:, l * D:(l + 1) * D],
                    start=False, stop=True)
                nc.scalar.activation(
                    v_sb[tt].rearrange("p (h e) -> p h e", e=VW)[:, :, 0:DH],
                    ps.rearrange("p (h e) -> p h e", e=DH),
                    AF.Identity, scale=edc_sb[:, tt:tt + 1])

            # ---- per head pair: Q, K, scores, exp, AV, normalize ----
            # o tiles share the vh_8..15 tags: h tiles of the previous FFN
            # are dead by the time attention writes o, and vice versa.
            o_sb = []
            for hp in range(NHP):
                ot = vhpool.tile([P, NTOK], BF, name=f"o_{hp}",
                                 tag=f"vh_{8 + hp}")
                o_sb.append(ot)
            pending = []

            def flush():
                if pending:
                    pending.pop(0)()

            def qk_proj(hp):
                wq_p = wpool.tile([P, NDT, P], BF, name="wq_p", tag="w")
                nc.sync.dma_start(
                    wq_p, wq_d[l, hp].rearrange("(t p) m -> p t m", p=P))
                wk_p = wpool.tile([P, NDT, P], BF, name="wk_p", tag="w")
                nc.sync.dma_start(
                    wk_p, wk_d[l, hp].rearrange("(t p) m -> p t m", p=P))
                q_p = qkpool.tile([P, NTOK], BF, name="q_p", tag="qk")
                k_p = qkpool.tile([P, NTOK], BF, name="k_p", tag="qk")
                for wt, dst, bias in ((wq_p, q_p, bq_sb[l]),
                                      (wk_p, k_p, bk_sb[l])):
                    ps = psum.tile([P, NTOK], F32, name="ps_qk", tag="ps")
                    for dt in range(NDT):
                        mm2(ps, wt[:, dt, :], xb_sb[dt],
                            start=(dt == 0), stop=(dt == NDT - 1))
                    nc.scalar.activation(dst, ps, AF.Identity,
                                         bias=bias[:, hp:hp + 1])
                return q_p, k_p

            qk_next = qk_proj(0)
            for hp in range(NHP):
                q_p, k_p = qk_next
                if hp + 1 < NHP:
                    qk_next = qk_proj(hp + 1)
                for b in range(BPC):
                    cs = slice(b * S, (b + 1) * S)
                    for lh in range(2):
                        h = hp * 2 + lh
                        rsl = slice(lh * DH, (lh + 1) * DH)
                        et = etpool.tile([P, 2 * NTOK], BF, name="et",
                                         tag="et")
                        for half in range(2):
                            ps = psum.tile([P, NTOK], F32, name="ps_sc",
                                           tag="ps")
                            for j in range(2):
                                st = half * 2 + j
                                nc.tensor.matmul(
                                    ps[:, j * S:(j + 1) * S],
                                    k_p[rsl,
                                        b * S + st * P: b * S + (st + 1) * P],
                                    q_p[rsl, cs])
                            nc.scalar.activation(
                                et[:, half * NTOK:(half + 1) * NTOK], ps,
                                AF.Exp, scale=stau_sb[:, b:b + 1])

                        def s2(et=et, ot=o_sb[hp], cs=cs, h=h, b=b, rsl=rsl):
                            pav = psum.tile([P, NTOK], F32, name="pav",
                                            tag="ps")
                            for st in range(NST):
                                nc.tensor.matmul(
                                    pav[0:VW, 0:S],
                                    v_sb[b * NST + st][:, h * VW:(h + 1) * VW],
                                    et[:, st * S:(st + 1) * S],
                                    start=(st == 0), stop=(st == NST - 1))
                            den_r = rowpool.tile([1, S], FR, name="den_r",
                                                 tag="den", bufs=2)
                            with nc.allow_low_precision(
                                    reason="f32r rows feed matmuls"):
                                nc.vector.reciprocal(den_r, pav[DH:DH + 1, 0:S])
                            # broadcast lands in the pav tile's second bank
                            # (cols S:2S, partitions 0:64). DVE can only
                            # read one PSUM operand, so bounce the broadcast
                            # through SBUF.
                            nc.tensor.matmul(pav[0:DH, S:2 * S],
                                             ones_row_fr[:, :DH], den_r)
                            rs_b = tmppool.tile([P, S], BF, name="rs_b",
                                                tag="tmp")
                            nc.scalar.activation(rs_b[0:DH, :],
                                                 pav[0:DH, S:2 * S], AF.Copy)
                            nc.vector.tensor_mul(ot[rsl, cs], pav[0:DH, 0:S],
                                                 rs_b[0:DH, :])

                        pending.append(s2)
                        if len(pending) > 1:
                            flush()
            while pending:
                flush()

            # ---- Wo projection + bias + residual (into x_sb f32r) ----
            for dto in range(NDT):
                wo_p = wpool.tile([P, NDT, P], BF, name="wo_p", tag="w")
                nc.sync.dma_start(
                    wo_p, wo_d[l, dto].rearrange("(t p) m -> p t m", p=P))
                ps = psum.tile([P, NTOK], F32, name="ps_wo", tag="ps")
                for dt in range(NDT):
                    mm2(ps, wo_p[:, dt, :], o_sb[dt],
                        start=(dt == 0), stop=(dt == NDT - 1))
                with nc.allow_low_precision(reason="residual in f32r"):
                    nc.vector.scalar_tensor_tensor(
                        x_sb[dto], ps, bo_sb[l][:, dto:dto + 1], xb_sb[dto],
                        ALU.add, ALU.add)

        def ffn_phase(l):
            # ---- h = gelu(W1 z + b1), all 32 f-tiles resident ----
            h_sb = []
            for ft in range(NFT):
                w1_p = wpool.tile([P, NDT, P], BF, name="w1_p", tag="w")
                nc.sync.dma_start(w1_p, w1_d[l, ft])
                ps = psum.tile([P, NTOK], F32, name="ps_h", tag="ps")
                for dt in range(NDT):
                    mm2(ps, w1_p[:, dt, :], xb_sb[dt],
                        start=(dt == 0), stop=(dt == NDT - 1))
                ht = vhpool.tile([P, NTOK], BF, name="htile", tag=f"vh_{ft}")
                nc.scalar.activation(ht, ps, gelu_f,
                                     bias=b1_sb[l][:, ft:ft + 1])
                h_sb.append(ht)
            # ---- y = W2 h (full-F PSUM accumulation, 2 waves of 4 dto) ----
            for half in range(2):
                ys = []
                for j in range(4):
                    yp = psum.tile([P, NTOK], F32, name="ps_y", tag="ps")
                    ys.append(yp)
                for ft in range(NFT):
                    w2_p = wpool.tile([P, 512], BF, name="w2_p", tag="w")
                    nc.sync.dma_start(w2_p, w2_d[l, half, ft])
                    for j in range(4):
                        mm2(ys[j], w2_p[:, j * P:(j + 1) * P], h_sb[ft],
                            start=(ft == 0), stop=(ft == NFT - 1))
                for j in range(4):
                    dto = half * 4 + j
                    with nc.allow_low_precision(reason="residual in f32r"):
                        nc.vector.scalar_tensor_tensor(
                            x_sb[dto], ys[j], b2_sb[l][:, dto:dto + 1],
                            xb_sb[dto], ALU.add, ALU.add)

        for l in range(L):
            attn_phase(l)
            ln(x_sb, g1_sb[l], be1_sb[l])
            ffn_phase(l)
            ln(x_sb, g2_sb[l], be2_sb[l])

        # ---- final LN + Wp (2 waves of 4 dto, dt-outer so matmuls start
        # as soon as the first normalized xb tile lands) ----
        # LNf consumes the LN2 output (xb), not the raw residual.
        ln(xb_sb, gf_sb, bf_sb)
        for half in range(2):
            wps, pss = [], []
            for j in range(4):
                dto = half * 4 + j
                wp_p = wpool.tile([P, NDT, P], BF, name="wp_p", tag="w")
                nc.sync.dma_start(
                    wp_p, wp_d[dto].rearrange("(t p) m -> p t m", p=P))
                wps.append(wp_p)
                pss.append(psum.tile([P, NTOK], F32, name="ps_wp", tag="ps"))
            for dt in range(NDT):
                for j in range(4):
                    mm2(pss[j], wps[j][:, dt, :], xb_sb[dt],
                        start=(dt == 0), stop=(dt == NDT - 1))
            for j in range(4):
                dto = half * 4 + j
                op = outpool.tile([P, NTOK], F32, name="outp", tag="out",
                                  bufs=2)
                nc.scalar.activation(op, pss[j], AF.Identity,
                                     bias=bp_sb[:, dto:dto + 1])
                for b in range(BPC):
                    nc.sync.dma_start(out_d[b, dto * P:(dto + 1) * P, :],
                                      op[:, b * S:(b + 1) * S])

    if reps == 1:
        body()
    else:
        with tc.For_i(0, reps, 1) as i:
            body(i)
    ctx.close()


# ======================= host side =======================

def _prep_core_inputs(inputs):
    """Build the 8 per-core input maps (weights shared, x/tau/delta sharded)."""
    import ml_dtypes
    bf = ml_dtypes.bfloat16
    f = np.float32
    x = np.asarray(inputs["x"], f)
    tau = np.asarray(inputs["tau"], f)
    delta = np.asarray(inputs["delta"], f)
    scale = 1.0 / np.sqrt(np.float32(DH))

    wq = np.asarray(inputs["Wq"], f)
    wk = np.asarray(inputs["Wk"], f)
    wv = np.asarray(inputs["Wv"], f)
    wo = np.asarray(inputs["Wo"], f)
    w1 = np.asarray(inputs["W1"], f)
    w2 = np.asarray(inputs["W2"], f)
    wp = np.asarray(inputs["Wp"], f)

    def hp_tiled(wt):  # [L, din, dout] -> [L, NHP, din, P]
        return np.ascontiguousarray(
            wt.reshape(L, D, NHP, P).transpose(0, 2, 1, 3)).astype(bf)

    wq_t = hp_tiled(wq.transpose(0, 2, 1))
    wk_t = hp_tiled(wk.transpose(0, 2, 1))
    wo_t = hp_tiled(wo.transpose(0, 2, 1))
    wv_t = np.ascontiguousarray(wv.transpose(0, 2, 1)).astype(bf)
    # W1 [L, F, D] -> W1^T [L, D, F] -> [L, NFT, P(d), NDT, P(f)]
    w1_t = np.ascontiguousarray(
        w1.transpose(0, 2, 1).reshape(L, NDT, P, NFT, P)
        .transpose(0, 3, 2, 1, 4)).astype(bf)
    # W2 [L, D, F] -> W2^T [L, F, D] -> [L, 2, NFT, P(f), 512(dto cols)]
    w2_t = np.ascontiguousarray(
        w2.transpose(0, 2, 1).reshape(L, NFT, P, 2, 512)
        .transpose(0, 3, 1, 2, 4)).astype(bf)
    wp_t = np.ascontiguousarray(
        wp.transpose(1, 0).reshape(D, NDT, P).transpose(1, 0, 2)).astype(bf)

    shared = {
        "wq_t": wq_t, "wk_t": wk_t, "wv_t": wv_t, "wo_t": wo_t,
        "w1_t": w1_t, "w2_t": w2_t, "wp_t": wp_t,
        "bv_bf": np.asarray(inputs["bv"], f).astype(bf),
    }

    def pcol(v):  # (n*P,) -> [P, n]
        v = np.asarray(v, f).reshape(-1, P)
        return v.T

    base_cols = []
    for l in range(L):
        for k in ("bq", "bk", "bo", "b2", "g1", "be1", "g2", "be2", "b1"):
            base_cols.append(pcol(inputs[k][l]))
    for k in ("gf", "bf", "bp"):
        base_cols.append(pcol(inputs[k]))

    in_maps = []
    for c in range(NCORES):
        bs = slice(c * BPC, (c + 1) * BPC)
        m = dict(shared)
        m["x_bf"] = np.ascontiguousarray(
            x[bs].transpose(0, 2, 1)).astype(bf)
        stau = np.tile((tau[bs] * scale).reshape(1, BPC), (P, 1))
        ed = np.exp(delta[bs] * scale).astype(f)          # [BPC, S]
        edc = np.ascontiguousarray(
            ed.reshape(BPC, NST, P).transpose(2, 0, 1).reshape(P, NDT))
        edr = np.repeat(edc[:, :, None], H, axis=2).reshape(P, NDT * H)
        m["pcols"] = np.ascontiguousarray(
            np.concatenate(base_cols + [stau, edc, edr], axis=1)).astype(f)
        in_maps.append(m)
    return in_maps


def run(inputs, reps=1):
    nc = _build(reps)
    in_maps = _prep_core_inputs(inputs)
    res = bass_utils.run_bass_kernel_spmd(nc, in_maps,
                                          core_ids=list(range(NCORES)))
    outs = [res.results[c]["out_fm"].transpose(0, 2, 1) for c in range(NCORES)]
    return np.ascontiguousarray(np.concatenate(outs, axis=0))


def kernel(**inputs) -> np.ndarray:
    return run(inputs, reps=1)


# revision 23
# speedup vs baseline: 3.2242x; 1.0289x over previous
"""Trainium2 Bass kernel for a 2-layer de-stationary-attention transformer.

Model (per reference):
  L=2 layers of: x += DSAttn(x); x = LN1(x); x = LN2(x + FFN(x)); then
  final LN + output projection Wp.
  DSAttn: softmax(scale * (Q K^T * tau + delta)) V with per-batch tau,
  per-(batch, key) delta.

Shapes: B=16, S=512, D=1024, H=16 heads (dh=64), F=4096.

Sharding: data-parallel over batch across 8 NeuronCores (2 batches/core),
weights replicated. No collectives.

v2 design notes:
  - All matmul operands bf16 (weights converted on host -> half the DMA
    bytes); fp32 PSUM accumulation; residual stream kept in f32r.
  - bf16 moving operands run at N=1024 (both batches per instruction),
    halving matmul instruction count vs fp32.
  - delta is folded into V: exp(scale*delta) scales V's columns (and
    replaces the ones-column that produces the softmax denominator), so
    exp(scores) needs only the per-batch tau scale -> one big ACT exp per
    score block instead of one per (key-tile).
  - LayerNorm: PE column-sum stats, rstd via Ln+Exp (stays in the exp
    table set), mean/rstd broadcast by K=1 matmuls then copied to SBUF so
    the per-tile normalize runs as two bf16 DVE ops at 2x rate.
  - FFN: all 32 h-tiles materialized in SBUF; y accumulated over the full
    F dimension in PSUM (two 4-d-tile waves x 8 banks); bias + residual
    fused into one scalar_tensor_tensor per output tile.
  - Residual adds fused with biases via scalar_tensor_tensor reading the
    matmul PSUM directly.
"""

import sys

if "/opt/trn_rl_repo" not in sys.path:
    sys.path.insert(0, "/opt/trn_rl_repo")

import numpy as np

import concourse.bass as bass
import concourse.bacc as bacc
import concourse.tile as tile
import concourse.mybir as mybir
from concourse import bass_utils

# Model dims
L, D, H, F = 2, 1024, 16, 4096
B, S = 16, 512
DH = D // H  # 64
NCORES = 8
BPC = B // NCORES   # batches per core
P = 128
NDT = D // P        # 8 d-tiles
NST = S // P        # 4 key-tiles per batch
NTOK = BPC * S      # 1024 tokens per core
NHP = H // 2        # 8 head pairs
NFT = F // P        # 32 f-tiles
VW = DH + 1         # 65: value width per head incl. denominator column
EPS = 1e-5

F32 = mybir.dt.float32
FR = mybir.dt.float32r
BF = mybir.dt.bfloat16
AF = mybir.ActivationFunctionType
ALU = mybir.AluOpType

_CACHE: dict = {}
import os
KGELU = os.environ.get("KGELU", "gelu")


def _build(reps: int):
    key = (reps, KGELU)
    if key in _CACHE:
        return _CACHE[key]

    nc = bacc.Bacc("TRN2", target_bir_lowering=False, debug=False,
                   num_devices=NCORES)

    # ---- DRAM tensors (per-core shapes) ----
    x_d = nc.dram_tensor("x_bf", (BPC, D, S), BF, kind="ExternalInput")
    wqk_d = nc.dram_tensor("wqk_t", (L, NHP, 2, D, P), BF,
                           kind="ExternalInput")
    wv_d = nc.dram_tensor("wv_t", (L, D, D), BF, kind="ExternalInput")
    wo_d = nc.dram_tensor("wo_t", (L, NDT, D, P), BF, kind="ExternalInput")
    w1_d = nc.dram_tensor("w1_t", (L, NFT, P, NDT, P), BF, kind="ExternalInput")
    w2_d = nc.dram_tensor("w2_t", (L, 2, NFT, P, 512), BF, kind="ExternalInput")
    wp_d = nc.dram_tensor("wp_t", (NDT, D, P), BF, kind="ExternalInput")
    bv_d = nc.dram_tensor("bv_bf", (L, D), BF, kind="ExternalInput")

    # all [P, 1]-sliceable f32 params packed into one column array:
    # per l: bq 8 | bk 8 | bo 8 | b2 8 | g1 8 | be1 8 | g2 8 | be2 8 | b1 32
    # then gf 8 | bf 8 | bp 8 | stau 2 | edc 8 | edr 128
    NPC = 96 * L + 8 * 3 + BPC + NDT + NDT * H
    pc_d = nc.dram_tensor("pcols", (P, NPC), F32, kind="ExternalInput")

    out_d = nc.dram_tensor("out_fm", (BPC, D, S), F32, kind="ExternalOutput")

    with tile.TileContext(nc) as tc:
        _emit(nc, tc, reps, locals())

    nc.compile()
    _CACHE[key] = nc
    return nc


def _emit(nc, tc, reps, d):
    x_d, wqk_d, wv_d, wo_d, w1_d, w2_d, wp_d = (
        d["x_d"], d["wqk_d"], d["wv_d"], d["wo_d"], d["w1_d"],
        d["w2_d"], d["wp_d"])
    bv_d, pc_d, out_d, NPC = d["bv_d"], d["pc_d"], d["out_d"], d["NPC"]

    from contextlib import ExitStack
    ctx = ExitStack()
    singles = ctx.enter_context(tc.tile_pool(name="singles", bufs=1))
    xpool = ctx.enter_context(tc.tile_pool(name="xpool", bufs=1))
    xbpool = ctx.enter_context(tc.tile_pool(name="xbpool", bufs=1))
    vhpool = ctx.enter_context(tc.tile_pool(name="vhpool", bufs=1))
    qkpool = ctx.enter_context(tc.tile_pool(name="qkpool", bufs=4))
    etpool = ctx.enter_context(tc.tile_pool(name="etpool", bufs=2))
    wpool = ctx.enter_context(tc.tile_pool(name="wpool", bufs=8))
    tmppool = ctx.enter_context(tc.tile_pool(name="tmppool", bufs=4))
    outpool = ctx.enter_context(tc.tile_pool(name="outpool", bufs=1))
    rowpool = ctx.enter_context(tc.tile_pool(name="rowpool", bufs=4))
    psum = ctx.enter_context(tc.tile_pool(name="psum", bufs=4, space="PSUM"))

    # ---- constants / params (loaded once, outside the reps loop) ----
    ones_f = singles.tile([P, 1], F32)
    nc.vector.memset(ones_f, 1.0)
    ones_col_fr = singles.tile([P, 1], FR)
    nc.scalar.activation(ones_col_fr, ones_f, AF.Copy)
    ones_col_bf = singles.tile([P, 1], BF)
    nc.scalar.activation(ones_col_bf, ones_f, AF.Copy)
    ones_rowf = singles.tile([1, P], F32)
    nc.vector.memset(ones_rowf, 1.0)
    ones_row_fr = singles.tile([1, P], FR)
    nc.scalar.activation(ones_row_fr, ones_rowf, AF.Copy)
    ones_row_bf = singles.tile([1, P], BF)
    nc.scalar.activation(ones_row_bf, ones_rowf, AF.Copy)
    eps_row = singles.tile([1, 1], F32)
    nc.vector.memset(eps_row, EPS)

    pc_sb = singles.tile([P, NPC], F32)
    nc.sync.dma_start(pc_sb, pc_d.ap())
    bv_sb = singles.tile([1, L * D], BF)
    nc.sync.dma_start(bv_sb, bv_d.ap().rearrange("l d -> (l d)")[None, :])

    _off = [0]

    def cols(n):
        c = pc_sb[:, _off[0]:_off[0] + n]
        _off[0] += n
        return c

    bq_sb, bk_sb, bo_sb, b2_sb = [], [], [], []
    g1_sb, be1_sb, g2_sb, be2_sb, b1_sb = [], [], [], [], []
    for l in range(L):
        bq_sb.append(cols(NHP))
        bk_sb.append(cols(NHP))
        bo_sb.append(cols(NDT))
        b2_sb.append(cols(NDT))
        g1_sb.append(cols(NDT))
        be1_sb.append(cols(NDT))
        g2_sb.append(cols(NDT))
        be2_sb.append(cols(NDT))
        b1_sb.append(cols(NFT))
    gf_sb = cols(NDT)
    bf_sb = cols(NDT)
    bp_sb = cols(NDT)
    stau_sb = cols(BPC)
    edc_sb = cols(NDT)
    edr_sb = cols(NDT * H)

    gelu_f = AF.Gelu if KGELU == "gelu" else AF.Identity

    def mm2(out, lhsT, rhs, start, stop):
        """Matmul with N=1024 moving operand split into two N=512 halves
        (matmul output must stay within one PSUM bank)."""
        for h2 in range(2):
            fs = slice(h2 * S, (h2 + 1) * S)
            nc.tensor.matmul(out[:, fs], lhsT, rhs[:, fs],
                             start=start, stop=stop)

    def body(_i=None):
        # ---- load x (feature-major, bf16) ----
        x_sb = []   # residual stream, bf16
        xb_sb = []  # normalized bf16 matmul operands
        for dt in range(NDT):
            xt = xpool.tile([P, NTOK], BF, name=f"x_{dt}", tag=f"x_{dt}")
            x_sb.append(xt)
            xbt = xbpool.tile([P, NTOK], BF, name=f"xb_{dt}", tag=f"xb_{dt}")
            nc.sync.dma_start(
                xbt.rearrange("p (b s) -> p b s", b=BPC),
                x_d.ap()[:, dt * P:(dt + 1) * P, :].rearrange(
                    "b p s -> p b s"))
            xb_sb.append(xbt)

        def ln(src, g_t, be_t):
            """LayerNorm over d (partitions): src = 8 tiles [P, NTOK] (f32r
            residual or bf16 xb); writes normalized bf16 into xb_sb. Stats
            for both batches land in one PSUM row [1, NTOK];
            rstd = exp(-0.5*ln(var+eps)) keeps ACT in the exp table set."""
            ps_s = psum.tile([P, NTOK], F32, name="ps_s", tag="ps")
            for dt in range(NDT):
                mm2(ps_s[0:1, :], ones_col_bf, src[dt],
                    start=(dt == 0), stop=(dt == NDT - 1))
            ps_q = psum.tile([P, NTOK], F32, name="ps_q", tag="ps")
            for dt in range(NDT):
                sq = tmppool.tile([P, NTOK], BF, name="sq", tag="tmp")
                nc.vector.tensor_mul(sq, src[dt], src[dt])
                mm2(ps_q[0:1, :], ones_col_bf, sq,
                    start=(dt == 0), stop=(dt == NDT - 1))
            mean_n = rowpool.tile([1, NTOK], FR, name="mean_n", tag="row")
            nc.vector.tensor_scalar(mean_n, ps_s[0:1, :], -1.0 / D, None,
                                    ALU.mult)
            m2 = rowpool.tile([1, NTOK], F32, name="m2", tag="row")
            nc.vector.tensor_mul(m2, mean_n, mean_n)
            var = rowpool.tile([1, NTOK], F32, name="var", tag="row")
            nc.vector.scalar_tensor_tensor(var, ps_q[0:1, :], 1.0 / D, m2,
                                           ALU.mult, ALU.subtract)
            lnv = rowpool.tile([1, NTOK], F32, name="lnv", tag="row")
            nc.scalar.activation(lnv, var, AF.Ln, bias=eps_row)
            rstd = rowpool.tile([1, NTOK], FR, name="rstd", tag="row")
            nc.scalar.activation(rstd, lnv, AF.Exp, scale=-0.5)
            # broadcast -mean and rstd across partitions, then to SBUF bf16
            pm = psum.tile([P, NTOK], F32, name="pm", tag="ps")
            pr = psum.tile([P, NTOK], F32, name="pr", tag="ps")
            for b in range(BPC):
                cs = slice(b * S, (b + 1) * S)
                nc.tensor.matmul(pm[:, cs], ones_row_fr, mean_n[:, cs])
                nc.tensor.matmul(pr[:, cs], ones_row_fr, rstd[:, cs])
            mb = tmppool.tile([P, NTOK], BF, name="mb", tag="mb", bufs=1)
            nc.scalar.activation(mb, pm, AF.Copy)
            rb = tmppool.tile([P, NTOK], BF, name="rb", tag="rb", bufs=1)
            nc.scalar.activation(rb, pr, AF.Copy)
            for dt in range(NDT):
                t1 = tmppool.tile([P, NTOK], BF, name="t1", tag="tmp")
                nc.vector.tensor_add(t1, src[dt], mb)
                t2 = tmppool.tile([P, NTOK], BF, name="t2", tag="tmp")
                nc.vector.tensor_mul(t2, t1, rb)
                nc.scalar.activation(xb_sb[dt], t2, AF.Identity,
                                     scale=g_t[:, dt:dt + 1],
                                     bias=be_t[:, dt:dt + 1])

        def attn_phase(l):
            # ---- V (token-major; denominator column = exp(delta)) ----
            wv_sb = []
            for dt in range(NDT):
                wt = wpool.tile([P, D], BF, name=f"wv_{dt}", tag="w")
                nc.sync.dma_start(wt, wv_d[l, dt * P:(dt + 1) * P, :])
                wv_sb.append(wt)
            v_sb = []
            for tt in range(NDT):
                vt = vhpool.tile([P, H * VW], BF, name=f"v_{tt}",
                                 tag=f"vh_{tt}")
                nc.scalar.activation(
                    vt.rearrange("p (h e) -> p h e", e=VW)[:, :, DH:DH + 1],
                    edr_sb[:, tt * H:(tt + 1) * H]
                    .rearrange("p (h o) -> p h o", o=1),
                    AF.Copy)
                v_sb.append(vt)
            for tt in range(NDT):
                ts = slice(tt * P, (tt + 1) * P)
                ps = psum.tile([P, NTOK], F32, name="ps_v", tag="ps")
                for dt in range(NDT):
                    mm2(ps, xb_sb[dt][:, ts], wv_sb[dt],
                        start=(dt == 0), stop=False)
                mm2(ps, ones_row_bf[:, :P], bv_sb[:, l * D:(l + 1) * D],
                    start=False, stop=True)
                nc.scalar.activation(
                    v_sb[tt].rearrange("p (h e) -> p h e", e=VW)[:, :, 0:DH],
                    ps.rearrange("p (h e) -> p h e", e=DH),
                    AF.Identity, scale=edc_sb[:, tt:tt + 1])

            # ---- per head pair: Q, K, scores, exp, AV, normalize ----
            # o tiles share the vh_8..15 tags: h tiles of the previous FFN
            # are dead by the time attention writes o, and vice versa.
            o_sb = []
            for hp in range(NHP):
                ot = vhpool.tile([P, NTOK], BF, name=f"o_{hp}",
                                 tag=f"vh_{8 + hp}")
                o_sb.append(ot)
            pending = []

            def qk_proj(hp):
                wqk_p = wpool.tile([P, 2, NDT, P], BF, name="wqk_p", tag="w2x",
                                   bufs=4)
                nc.sync.dma_start(
                    wqk_p, wqk_d[l, hp].rearrange("q (t p) m -> p q t m", p=P))
                q_p = qkpool.tile([P, NTOK], BF, name="q_p", tag="qk")
                k_p = qkpool.tile([P, NTOK], BF, name="k_p", tag="qk")
                for qi, (dst, bias) in enumerate(((q_p, bq_sb[l]),
                                                 (k_p, bk_sb[l]))):
                    ps = psum.tile([P, NTOK], F32, name="ps_qk", tag="ps")
                    for dt in range(NDT):
                        mm2(ps, wqk_p[:, qi, dt, :], xb_sb[dt],
                            start=(dt == 0), stop=(dt == NDT - 1))
                    nc.scalar.activation(dst, ps, AF.Identity,
                                         bias=bias[:, hp:hp + 1])
                return q_p, k_p

            def wo_proj(dto):
                # Wo column block dto consumes o_sb[dto] (written by head
                # pair dto's groups); interleaved into the hp loop with a
                # 2-hp lag to feed the PE during the ACT-heavy group loop.
                wo_p = wpool.tile([P, NDT, P], BF, name="wo_p", tag="w")
                nc.sync.dma_start(
                    wo_p, wo_d[l, dto].rearrange("(t p) m -> p t m", p=P))
                ps = psum.tile([P, NTOK], F32, name="ps_wo", tag="ps")
                for dt in range(NDT):
                    mm2(ps, wo_p[:, dt, :], o_sb[dt],
                        start=(dt == 0), stop=(dt == NDT - 1))
                nc.vector.scalar_tensor_tensor(
                    x_sb[dto], ps, bo_sb[l][:, dto:dto + 1], xb_sb[dto],
                    ALU.add, ALU.add)

            qk_next = qk_proj(0)
            for hp in range(NHP):
                q_p, k_p = qk_next
                if hp + 1 < NHP:
                    qk_next = qk_proj(hp + 1)
                for b in range(BPC):
                    cs = slice(b * S, (b + 1) * S)
                    for lh in range(2):
                        h = hp * 2 + lh
                        rsl = slice(lh * DH, (lh + 1) * DH)
                        # stage 2b of the group two iterations back runs
                        # first so its broadcast matmul is already queued
                        # when this group's score matmuls claim its slot.
                        if len(pending) > 1:
                            pending.pop(0)[1]()
                        et = etpool.tile([P, 2 * NTOK], BF, name="et",
                                         tag="et")
                        for half in range(2):
                            ps = psum.tile([P, NTOK], F32, name="ps_sc",
                                           tag="ps")
                            for j in range(2):
                                st = half * 2 + j
                                nc.tensor.matmul(
                                    ps[:, j * S:(j + 1) * S],
                                    k_p[rsl,
                                        b * S + st * P: b * S + (st + 1) * P],
                                    q_p[rsl, cs])
                            nc.scalar.activation(
                                et[:, half * NTOK:(half + 1) * NTOK], ps,
                                AF.Exp, scale=stau_sb[:, b:b + 1])

                        state = {}

                        def s2a(et=et, h=h, b=b, state=state):
                            pav = psum.tile([P, NTOK], F32, name="pav",
                                            tag="ps")
                            for st in range(NST):
                                nc.tensor.matmul(
                                    pav[0:VW, 0:S],
                                    v_sb[b * NST + st][:, h * VW:(h + 1) * VW],
                                    et[:, st * S:(st + 1) * S],
                                    start=(st == 0), stop=(st == NST - 1))
                            den_r = rowpool.tile([1, S], FR, name="den_r",
                                                 tag="den", bufs=2)
                            with nc.allow_low_precision(
                                    reason="f32r rows feed matmuls"):
                                nc.vector.reciprocal(den_r,
                                                     pav[DH:DH + 1, 0:S])
                            state["pav"] = pav
                            state["den_r"] = den_r

                        def s2b(ot=o_sb[hp], cs=cs, rsl=rsl, state=state):
                            pav, den_r = state["pav"], state["den_r"]
                            # broadcast lands in the pav tile's second bank
                            # (cols S:2S, partitions 0:64). DVE can only
                            # read one PSUM operand, so bounce the broadcast
                            # through SBUF.
                            nc.tensor.matmul(pav[0:DH, S:2 * S],
                                             ones_row_fr[:, :DH], den_r)
                            rs_b = tmppool.tile([P, S], BF, name="rs_b",
                                                tag="tmp")
                            nc.vector.tensor_copy(rs_b[0:DH, :],
                                                  pav[0:DH, S:2 * S])
                            nc.vector.tensor_mul(ot[rsl, cs],
                                                 pav[0:DH, 0:S],
                                                 rs_b[0:DH, :])

                        pending.append((s2a, s2b))
                        if len(pending) > 1:
                            pending[-2][0]()  # run previous group's s2a
            # drain: s2a of the last group, then remaining s2b's
            if pending:
                pending[-1][0]()
            while pending:
                pending.pop(0)[1]()
            for dto in range(NDT):
                wo_proj(dto)

        def ffn_phase(l):
            # ---- h = gelu(W1 z + b1), all 32 f-tiles resident ----
            h_sb = []
            for ft in range(NFT):
                w1_p = wpool.tile([P, NDT, P], BF, name="w1_p", tag="w")
                nc.sync.dma_start(w1_p, w1_d[l, ft])
                ps = psum.tile([P, NTOK], F32, name="ps_h", tag="ps")
                for dt in range(NDT):
                    mm2(ps, w1_p[:, dt, :], xb_sb[dt],
                        start=(dt == 0), stop=(dt == NDT - 1))
                ht = vhpool.tile([P, NTOK], BF, name="htile", tag=f"vh_{ft}")
                nc.scalar.activation(ht, ps, gelu_f,
                                     bias=b1_sb[l][:, ft:ft + 1])
                h_sb.append(ht)
            # ---- y = W2 h (full-F PSUM accumulation, 2 waves of 4 dto) ----
            for half in range(2):
                ys = []
                for j in range(4):
                    yp = psum.tile([P, NTOK], F32, name="ps_y", tag="ps")
                    ys.append(yp)
                for ft in range(NFT):
                    w2_p = wpool.tile([P, 512], BF, name="w2_p", tag="w")
                    nc.sync.dma_start(w2_p, w2_d[l, half, ft])
                    for j in range(4):
                        mm2(ys[j], w2_p[:, j * P:(j + 1) * P], h_sb[ft],
                            start=(ft == 0), stop=(ft == NFT - 1))
                for j in range(4):
                    dto = half * 4 + j
                    nc.vector.scalar_tensor_tensor(
                        x_sb[dto], ys[j], b2_sb[l][:, dto:dto + 1],
                        xb_sb[dto], ALU.add, ALU.add)

        for l in range(L):
            attn_phase(l)
            ln(x_sb, g1_sb[l], be1_sb[l])
            ffn_phase(l)
            ln(x_sb, g2_sb[l], be2_sb[l])

        # ---- final LN + Wp (2 waves of 4 dto, dt-outer so matmuls start
        # as soon as the first normalized xb tile lands) ----
        # LNf consumes the LN2 output (xb), not the raw residual.
        ln(xb_sb, gf_sb, bf_sb)
        for half in range(2):
            wps, pss = [], []
            for j in range(4):
                dto = half * 4 + j
                wp_p = wpool.tile([P, NDT, P], BF, name="wp_p", tag="w")
                nc.sync.dma_start(
                    wp_p, wp_d[dto].rearrange("(t p) m -> p t m", p=P))
                wps.append(wp_p)
                pss.append(psum.tile([P, NTOK], F32, name="ps_wp", tag="ps"))
            for dt in range(NDT):
                for j in range(4):
                    mm2(pss[j], wps[j][:, dt, :], xb_sb[dt],
                        start=(dt == 0), stop=(dt == NDT - 1))
            for j in range(4):
                dto = half * 4 + j
                op = outpool.tile([P, NTOK], F32, name="outp", tag="out",
                                  bufs=2)
                nc.scalar.activation(op, pss[j], AF.Identity,
                                     bias=bp_sb[:, dto:dto + 1])
                nc.sync.dma_start(
                    out_d.ap()[:, dto * P:(dto + 1) * P, :].rearrange(
                        "b p s -> p b s"),
                    op.rearrange("p (b s) -> p b s", b=BPC))

    if reps == 1:
        body()
    else:
        with tc.For_i(0, reps, 1) as i:
            body(i)
    ctx.close()


# ======================= host side =======================

def _prep_core_inputs(inputs):
    """Build the 8 per-core input maps (weights shared, x/tau/delta sharded)."""
    import ml_dtypes
    bf = ml_dtypes.bfloat16
    f = np.float32
    x = np.asarray(inputs["x"], f)
    tau = np.asarray(inputs["tau"], f)
    delta = np.asarray(inputs["delta"], f)
    scale = 1.0 / np.sqrt(np.float32(DH))

    wq = np.asarray(inputs["Wq"], f)
    wk = np.asarray(inputs["Wk"], f)
    wv = np.asarray(inputs["Wv"], f)
    wo = np.asarray(inputs["Wo"], f)
    w1 = np.asarray(inputs["W1"], f)
    w2 = np.asarray(inputs["W2"], f)
    wp = np.asarray(inputs["Wp"], f)

    def hp_tiled(wt):  # [L, din, dout] -> [L, NHP, din, P]
        return np.ascontiguousarray(
            wt.reshape(L, D, NHP, P).transpose(0, 2, 1, 3)).astype(bf)

    wq_t = hp_tiled(wq.transpose(0, 2, 1))
    wk_t = hp_tiled(wk.transpose(0, 2, 1))
    wqk_t = np.ascontiguousarray(np.stack([wq_t, wk_t], axis=2))
    wo_t = hp_tiled(wo.transpose(0, 2, 1))
    wv_t = np.ascontiguousarray(wv.transpose(0, 2, 1)).astype(bf)
    # W1 [L, F, D] -> W1^T [L, D, F] -> [L, NFT, P(d), NDT, P(f)]
    w1_t = np.ascontiguousarray(
        w1.transpose(0, 2, 1).reshape(L, NDT, P, NFT, P)
        .transpose(0, 3, 2, 1, 4)).astype(bf)
    # W2 [L, D, F] -> W2^T [L, F, D] -> [L, 2, NFT, P(f), 512(dto cols)]
    w2_t = np.ascontiguousarray(
        w2.transpose(0, 2, 1).reshape(L, NFT, P, 2, 512)
        .transpose(0, 3, 1, 2, 4)).astype(bf)
    wp_t = np.ascontiguousarray(
        wp.transpose(1, 0).reshape(D, NDT, P).transpose(1, 0, 2)).astype(bf)

    shared = {
        "wqk_t": wqk_t, "wv_t": wv_t, "wo_t": wo_t,
        "w1_t": w1_t, "w2_t": w2_t, "wp_t": wp_t,
        "bv_bf": np.asarray(inputs["bv"], f).astype(bf),
    }

    def pcol(v):  # (n*P,) -> [P, n]
        v = np.asarray(v, f).reshape(-1, P)
        return v.T

    base_cols = []
    for l in range(L):
        for k in ("bq", "bk", "bo", "b2", "g1", "be1", "g2", "be2", "b1"):
            base_cols.append(pcol(inputs[k][l]))
    for k in ("gf", "bf", "bp"):
        base_cols.append(pcol(inputs[k]))

    in_maps = []
    for c in range(NCORES):
        bs = slice(c * BPC, (c + 1) * BPC)
        m = dict(shared)
        m["x_bf"] = np.ascontiguousarray(
            x[bs].transpose(0, 2, 1)).astype(bf)
        stau = np.tile((tau[bs] * scale).reshape(1, BPC), (P, 1))
        ed = np.exp(delta[bs] * scale).astype(f)          # [BPC, S]
        edc = np.ascontiguousarray(
            ed.reshape(BPC, NST, P).transpose(2, 0, 1).reshape(P, NDT))
        edr = np.repeat(edc[:, :, None], H, axis=2).reshape(P, NDT * H)
        m["pcols"] = np.ascontiguousarray(
            np.concatenate(base_cols + [stau, edc, edr], axis=1)).astype(f)
        in_maps.append(m)
    return in_maps


def run(inputs, reps=1):
    nc = _build(reps)
    in_maps = _prep_core_inputs(inputs)
    res = bass_utils.run_bass_kernel_spmd(nc, in_maps,
                                          core_ids=list(range(NCORES)))
    outs = [res.results[c]["out_fm"].transpose(0, 2, 1) for c in range(NCORES)]
    return np.ascontiguousarray(np.concatenate(outs, axis=0))


def kernel(**inputs) -> np.ndarray:
    return run(inputs, reps=1)


# revision 25
# speedup vs baseline: 3.5377x; 1.0972x over previous
"""Trainium2 Bass kernel for a 2-layer de-stationary-attention transformer.

Model (per reference):
  L=2 layers of: x += DSAttn(x); x = LN1(x); x = LN2(x + FFN(x)); then
  final LN + output projection Wp.
  DSAttn: softmax(scale * (Q K^T * tau + delta)) V with per-batch tau,
  per-(batch, key) delta.

Shapes: B=16, S=512, D=1024, H=16 heads (dh=64), F=4096.

Sharding: data-parallel over batch across 8 NeuronCores (2 batches/core),
weights replicated. No collectives.

v2 design notes:
  - All matmul operands bf16 (weights converted on host -> half the DMA
    bytes); fp32 PSUM accumulation; residual stream kept in f32r.
  - bf16 moving operands run at N=1024 (both batches per instruction),
    halving matmul instruction count vs fp32.
  - delta is folded into V: exp(scale*delta) scales V's columns (and
    replaces the ones-column that produces the softmax denominator), so
    exp(scores) needs only the per-batch tau scale -> one big ACT exp per
    score block instead of one per (key-tile).
  - LayerNorm: PE column-sum stats, rstd via Ln+Exp (stays in the exp
    table set), mean/rstd broadcast by K=1 matmuls then copied to SBUF so
    the per-tile normalize runs as two bf16 DVE ops at 2x rate.
  - FFN: all 32 h-tiles materialized in SBUF; y accumulated over the full
    F dimension in PSUM (two 4-d-tile waves x 8 banks); bias + residual
    fused into one scalar_tensor_tensor per output tile.
  - Residual adds fused with biases via scalar_tensor_tensor reading the
    matmul PSUM directly.
"""

import sys

if "/opt/trn_rl_repo" not in sys.path:
    sys.path.insert(0, "/opt/trn_rl_repo")

import numpy as np

import concourse.bass as bass
import concourse.bacc as bacc
import concourse.tile as tile
import concourse.mybir as mybir
from concourse import bass_utils

# Model dims
L, D, H, F = 2, 1024, 16, 4096
B, S = 16, 512
DH = D // H  # 64
NCORES = 8
BPC = B // NCORES   # batches per core
P = 128
NDT = D // P        # 8 d-tiles
NST = S // P        # 4 key-tiles per batch
NTOK = BPC * S      # 1024 tokens per core
NHP = H // 2        # 8 head pairs
NFT = F // P        # 32 f-tiles
VW = DH + 1         # 65: value width per head incl. denominator column
EPS = 1e-5

F32 = mybir.dt.float32
FR = mybir.dt.float32r
BF = mybir.dt.bfloat16
AF = mybir.ActivationFunctionType
ALU = mybir.AluOpType

_CACHE: dict = {}
import os
KGELU = os.environ.get("KGELU", "gelu")


def _build(reps: int):
    key = (reps, KGELU)
    if key in _CACHE:
        return _CACHE[key]

    nc = bacc.Bacc("TRN2", target_bir_lowering=False, debug=False,
                   num_devices=NCORES)

    # ---- DRAM tensors (per-core shapes) ----
    x_d = nc.dram_tensor("x_bf", (BPC, D, S), BF, kind="ExternalInput")
    wqk_d = nc.dram_tensor("wqk_t", (L, NHP, 2, D, P), BF,
                           kind="ExternalInput")
    wv_d = nc.dram_tensor("wv_t", (L, D, D), BF, kind="ExternalInput")
    wo_d = nc.dram_tensor("wo_t", (L, NDT, D, P), BF, kind="ExternalInput")
    w1_d = nc.dram_tensor("w1_t", (L, NFT, P, NDT, P), BF, kind="ExternalInput")
    w2_d = nc.dram_tensor("w2_t", (L, 2, NFT, P, 512), BF, kind="ExternalInput")
    wp_d = nc.dram_tensor("wp_t", (NDT, D, P), BF, kind="ExternalInput")
    bv_d = nc.dram_tensor("bv_bf", (L, D), BF, kind="ExternalInput")

    # all [P, 1]-sliceable f32 params packed into one column array:
    # per l: bq 8 | bk 8 | bo 8 | b2 8 | g1 8 | be1 8 | g2 8 | be2 8 | b1 32
    # then gf 8 | bf 8 | bp 8 | stau 2 | edc 8 | edr 128
    NPC = 96 * L + 8 * 3 + BPC + NDT + NDT * H
    pc_d = nc.dram_tensor("pcols", (P, NPC), F32, kind="ExternalInput")

    out_d = nc.dram_tensor("out_fm", (BPC, D, S), F32, kind="ExternalOutput")

    with tile.TileContext(nc) as tc:
        _emit(nc, tc, reps, locals())

    nc.compile()
    _CACHE[key] = nc
    return nc


def _emit(nc, tc, reps, d):
    x_d, wqk_d, wv_d, wo_d, w1_d, w2_d, wp_d = (
        d["x_d"], d["wqk_d"], d["wv_d"], d["wo_d"], d["w1_d"],
        d["w2_d"], d["wp_d"])
    bv_d, pc_d, out_d, NPC = d["bv_d"], d["pc_d"], d["out_d"], d["NPC"]

    from contextlib import ExitStack
    ctx = ExitStack()
    singles = ctx.enter_context(tc.tile_pool(name="singles", bufs=1))
    xpool = ctx.enter_context(tc.tile_pool(name="xpool", bufs=1))
    xbpool = ctx.enter_context(tc.tile_pool(name="xbpool", bufs=1))
    vhpool = ctx.enter_context(tc.tile_pool(name="vhpool", bufs=1))
    qkpool = ctx.enter_context(tc.tile_pool(name="qkpool", bufs=4))
    etpool = ctx.enter_context(tc.tile_pool(name="etpool", bufs=2))
    wpool = ctx.enter_context(tc.tile_pool(name="wpool", bufs=8))
    tmppool = ctx.enter_context(tc.tile_pool(name="tmppool", bufs=4))
    outpool = ctx.enter_context(tc.tile_pool(name="outpool", bufs=1))
    rowpool = ctx.enter_context(tc.tile_pool(name="rowpool", bufs=4))
    psum = ctx.enter_context(tc.tile_pool(name="psum", bufs=4, space="PSUM"))

    # ---- constants / params (loaded once, outside the reps loop) ----
    ones_f = singles.tile([P, 1], F32)
    nc.vector.memset(ones_f, 1.0)
    ones_col_fr = singles.tile([P, 1], FR)
    nc.scalar.activation(ones_col_fr, ones_f, AF.Copy)
    ones_col_bf = singles.tile([P, 1], BF)
    nc.scalar.activation(ones_col_bf, ones_f, AF.Copy)
    ones_rowf = singles.tile([1, P], F32)
    nc.vector.memset(ones_rowf, 1.0)
    ones_row_fr = singles.tile([1, P], FR)
    nc.scalar.activation(ones_row_fr, ones_rowf, AF.Copy)
    ones_row_bf = singles.tile([1, P], BF)
    nc.scalar.activation(ones_row_bf, ones_rowf, AF.Copy)
    eps_row = singles.tile([1, 1], F32)
    nc.vector.memset(eps_row, EPS)

    pc_sb = singles.tile([P, NPC], F32)
    nc.sync.dma_start(pc_sb, pc_d.ap())
    bv_sb = singles.tile([1, L * D], BF)
    nc.sync.dma_start(bv_sb, bv_d.ap().rearrange("l d -> (l d)")[None, :])

    _off = [0]

    def cols(n):
        c = pc_sb[:, _off[0]:_off[0] + n]
        _off[0] += n
        return c

    bq_sb, bk_sb, bo_sb, b2_sb = [], [], [], []
    g1_sb, be1_sb, g2_sb, be2_sb, b1_sb = [], [], [], [], []
    for l in range(L):
        bq_sb.append(cols(NHP))
        bk_sb.append(cols(NHP))
        bo_sb.append(cols(NDT))
        b2_sb.append(cols(NDT))
        g1_sb.append(cols(NDT))
        be1_sb.append(cols(NDT))
        g2_sb.append(cols(NDT))
        be2_sb.append(cols(NDT))
        b1_sb.append(cols(NFT))
    gf_sb = cols(NDT)
    bf_sb = cols(NDT)
    bp_sb = cols(NDT)
    stau_sb = cols(BPC)
    edc_sb = cols(NDT)
    edr_sb = cols(NDT * H)

    gelu_f = AF.Gelu if KGELU == "gelu" else AF.Identity

    def mm2(out, lhsT, rhs, start, stop):
        """Matmul with N=1024 moving operand split into two N=512 halves
        (matmul output must stay within one PSUM bank)."""
        for h2 in range(2):
            fs = slice(h2 * S, (h2 + 1) * S)
            nc.tensor.matmul(out[:, fs], lhsT, rhs[:, fs],
                             start=start, stop=stop)

    def body(_i=None):
        # ---- load x (feature-major, bf16) ----
        x_sb = []   # residual stream, bf16
        xb_sb = []  # normalized bf16 matmul operands
        for dt in range(NDT):
            xt = xpool.tile([P, NTOK], BF, name=f"x_{dt}", tag=f"x_{dt}")
            x_sb.append(xt)
            xbt = xbpool.tile([P, NTOK], BF, name=f"xb_{dt}", tag=f"xb_{dt}")
            nc.sync.dma_start(
                xbt.rearrange("p (b s) -> p b s", b=BPC),
                x_d.ap()[:, dt * P:(dt + 1) * P, :].rearrange(
                    "b p s -> p b s"))
            xb_sb.append(xbt)

        def ln(src, g_t, be_t):
            """LayerNorm over d (partitions): src = 8 tiles [P, NTOK] (f32r
            residual or bf16 xb); writes normalized bf16 into xb_sb. Stats
            for both batches land in one PSUM row [1, NTOK];
            rstd = exp(-0.5*ln(var+eps)) keeps ACT in the exp table set."""
            ps_s = psum.tile([P, NTOK], F32, name="ps_s", tag="ps")
            for dt in range(NDT):
                mm2(ps_s[0:1, :], ones_col_bf, src[dt],
                    start=(dt == 0), stop=(dt == NDT - 1))
            ps_q = psum.tile([P, NTOK], F32, name="ps_q", tag="ps")
            for dt in range(NDT):
                sq = tmppool.tile([P, NTOK], BF, name="sq", tag="tmp")
                nc.vector.tensor_mul(sq, src[dt], src[dt])
                mm2(ps_q[0:1, :], ones_col_bf, sq,
                    start=(dt == 0), stop=(dt == NDT - 1))
            mean_n = rowpool.tile([1, NTOK], FR, name="mean_n", tag="row")
            nc.vector.tensor_scalar(mean_n, ps_s[0:1, :], -1.0 / D, None,
                                    ALU.mult)
            m2 = rowpool.tile([1, NTOK], F32, name="m2", tag="row")
            nc.vector.tensor_mul(m2, mean_n, mean_n)
            var = rowpool.tile([1, NTOK], F32, name="var", tag="row")
            nc.vector.scalar_tensor_tensor(var, ps_q[0:1, :], 1.0 / D, m2,
                                           ALU.mult, ALU.subtract)
            lnv = rowpool.tile([1, NTOK], F32, name="lnv", tag="row")
            nc.scalar.activation(lnv, var, AF.Ln, bias=eps_row)
            rstd = rowpool.tile([1, NTOK], FR, name="rstd", tag="row")
            nc.scalar.activation(rstd, lnv, AF.Exp, scale=-0.5)
            # broadcast -mean and rstd across partitions, then to SBUF bf16
            pm = psum.tile([P, NTOK], F32, name="pm", tag="ps")
            pr = psum.tile([P, NTOK], F32, name="pr", tag="ps")
            for b in range(BPC):
                cs = slice(b * S, (b + 1) * S)
                nc.tensor.matmul(pm[:, cs], ones_row_fr, mean_n[:, cs])
                nc.tensor.matmul(pr[:, cs], ones_row_fr, rstd[:, cs])
            mb = tmppool.tile([P, NTOK], BF, name="mb", tag="mb", bufs=1)
            nc.scalar.activation(mb, pm, AF.Copy)
            rb = tmppool.tile([P, NTOK], BF, name="rb", tag="rb", bufs=1)
            nc.scalar.activation(rb, pr, AF.Copy)
            for dt in range(NDT):
                t1 = tmppool.tile([P, NTOK], BF, name="t1", tag="tmp")
                nc.vector.tensor_add(t1, src[dt], mb)
                t2 = tmppool.tile([P, NTOK], BF, name="t2", tag="tmp")
                nc.vector.tensor_mul(t2, t1, rb)
                nc.scalar.activation(xb_sb[dt], t2, AF.Identity,
                                     scale=g_t[:, dt:dt + 1],
                                     bias=be_t[:, dt:dt + 1])

        def attn_phase(l):
            # ---- V (token-major; denominator column = exp(delta)) ----
            wv_sb = []
            for dt in range(NDT):
                wt = wpool.tile([P, D], BF, name=f"wv_{dt}", tag="w")
                nc.sync.dma_start(wt, wv_d[l, dt * P:(dt + 1) * P, :])
                wv_sb.append(wt)
            v_sb = []
            for tt in range(NDT):
                vt = vhpool.tile([P, H * VW], BF, name=f"v_{tt}",
                                 tag=f"vh_{tt}")
                nc.scalar.activation(
                    vt.rearrange("p (h e) -> p h e", e=VW)[:, :, DH:DH + 1],
                    edr_sb[:, tt * H:(tt + 1) * H]
                    .rearrange("p (h o) -> p h o", o=1),
                    AF.Copy)
                v_sb.append(vt)
            for tt in range(NDT):
                ts = slice(tt * P, (tt + 1) * P)
                ps = psum.tile([P, NTOK], F32, name="ps_v", tag="ps")
                for dt in range(NDT):
                    mm2(ps, xb_sb[dt][:, ts], wv_sb[dt],
                        start=(dt == 0), stop=False)
                mm2(ps, ones_row_bf[:, :P], bv_sb[:, l * D:(l + 1) * D],
                    start=False, stop=True)
                nc.scalar.activation(
                    v_sb[tt].rearrange("p (h e) -> p h e", e=VW)[:, :, 0:DH],
                    ps.rearrange("p (h e) -> p h e", e=DH),
                    AF.Identity, scale=edc_sb[:, tt:tt + 1])

            # ---- per head pair: Q, K, scores, exp, AV, normalize ----
            # o tiles share the vh_8..15 tags: h tiles of the previous FFN
            # are dead by the time attention writes o, and vice versa.
            o_sb = []
            for hp in range(NHP):
                ot = vhpool.tile([P, NTOK], BF, name=f"o_{hp}",
                                 tag=f"vh_{8 + hp}")
                o_sb.append(ot)
            pending = []

            def qk_proj(hp):
                wqk_p = wpool.tile([P, 2, NDT, P], BF, name="wqk_p", tag="w2x",
                                   bufs=4)
                nc.sync.dma_start(
                    wqk_p, wqk_d[l, hp].rearrange("q (t p) m -> p q t m", p=P))
                q_p = qkpool.tile([P, NTOK], BF, name="q_p", tag="qk")
                k_p = qkpool.tile([P, NTOK], BF, name="k_p", tag="qk")
                for qi, (dst, bias) in enumerate(((q_p, bq_sb[l]),
                                                 (k_p, bk_sb[l]))):
                    ps = psum.tile([P, NTOK], F32, name="ps_qk", tag="ps")
                    for dt in range(NDT):
                        mm2(ps, wqk_p[:, qi, dt, :], xb_sb[dt],
                            start=(dt == 0), stop=(dt == NDT - 1))
                    nc.scalar.activation(dst, ps, AF.Identity,
                                         bias=bias[:, hp:hp + 1])
                return q_p, k_p

            def wo_proj(dto):
                # Wo column block dto consumes o_sb[dto] (written by head
                # pair dto's groups); interleaved into the hp loop with a
                # 2-hp lag to feed the PE during the ACT-heavy group loop.
                wo_p = wpool.tile([P, NDT, P], BF, name="wo_p", tag="w")
                nc.sync.dma_start(
                    wo_p, wo_d[l, dto].rearrange("(t p) m -> p t m", p=P))
                ps = psum.tile([P, NTOK], F32, name="ps_wo", tag="ps")
                for dt in range(NDT):
                    mm2(ps, wo_p[:, dt, :], o_sb[dt],
                        start=(dt == 0), stop=(dt == NDT - 1))
                nc.vector.scalar_tensor_tensor(
                    x_sb[dto], ps, bo_sb[l][:, dto:dto + 1], xb_sb[dto],
                    ALU.add, ALU.add)

            qk_next = qk_proj(0)
            for hp in range(NHP):
                q_p, k_p = qk_next
                if hp + 1 < NHP:
                    qk_next = qk_proj(hp + 1)
                for b in range(BPC):
                    cs = slice(b * S, (b + 1) * S)
                    for lh in range(2):
                        h = hp * 2 + lh
                        rsl = slice(lh * DH, (lh + 1) * DH)
                        # stage 2b of the group two iterations back runs
                        # first so its broadcast matmul is already queued
                        # when this group's score matmuls claim its slot.
                        if len(pending) > 1:
                            pending.pop(0)[1]()
                        et = etpool.tile([P, 2 * NTOK], BF, name="et",
                                         tag="et")
                        for half in range(2):
                            ps = psum.tile([P, NTOK], F32, name="ps_sc",
                                           tag="ps")
                            for j in range(2):
                                st = half * 2 + j
                                nc.tensor.matmul(
                                    ps[:, j * S:(j + 1) * S],
                                    k_p[rsl,
                                        b * S + st * P: b * S + (st + 1) * P],
                                    q_p[rsl, cs])
                            nc.scalar.activation(
                                et[:, half * NTOK:(half + 1) * NTOK], ps,
                                AF.Exp, scale=stau_sb[:, b:b + 1])

                        state = {}

                        def s2a(et=et, h=h, b=b, state=state):
                            pav = psum.tile([P, NTOK], F32, name="pav",
                                            tag="ps")
                            for st in range(NST):
                                nc.tensor.matmul(
                                    pav[0:VW, 0:S],
                                    v_sb[b * NST + st][:, h * VW:(h + 1) * VW],
                                    et[:, st * S:(st + 1) * S],
                                    start=(st == 0), stop=(st == NST - 1))
                            den_r = rowpool.tile([1, S], FR, name="den_r",
                                                 tag="den", bufs=2)
                            with nc.allow_low_precision(
                                    reason="f32r rows feed matmuls"):
                                nc.vector.reciprocal(den_r,
                                                     pav[DH:DH + 1, 0:S])
                            state["pav"] = pav
                            state["den_r"] = den_r

                        def s2b(ot=o_sb[hp], cs=cs, rsl=rsl, state=state):
                            pav, den_r = state["pav"], state["den_r"]
                            # broadcast lands in the pav tile's second bank
                            # (cols S:2S, partitions 0:64). DVE can only
                            # read one PSUM operand, so bounce the broadcast
                            # through SBUF.
                            nc.tensor.matmul(pav[0:DH, S:2 * S],
                                             ones_row_fr[:, :DH], den_r)
                            rs_b = tmppool.tile([P, S], BF, name="rs_b",
                                                tag="tmp")
                            nc.vector.tensor_copy(rs_b[0:DH, :],
                                                  pav[0:DH, S:2 * S])
                            nc.vector.tensor_mul(ot[rsl, cs],
                                                 pav[0:DH, 0:S],
                                                 rs_b[0:DH, :])

                        pending.append((s2a, s2b))
                        if len(pending) > 1:
                            pending[-2][0]()  # run previous group's s2a
            # drain: s2a of the last group, then remaining s2b's
            if pending:
                pending[-1][0]()
            while pending:
                pending.pop(0)[1]()
            for dto in range(NDT):
                wo_proj(dto)

        def ffn_phase(l):
            # ---- h = gelu(W1 z + b1), all 32 f-tiles resident ----
            h_sb = []
            for ft in range(NFT):
                w1_p = wpool.tile([P, NDT, P], BF, name="w1_p", tag="w")
                nc.sync.dma_start(w1_p, w1_d[l, ft])
                ps = psum.tile([P, NTOK], F32, name="ps_h", tag="ps")
                for dt in range(NDT):
                    mm2(ps, w1_p[:, dt, :], xb_sb[dt],
                        start=(dt == 0), stop=(dt == NDT - 1))
                ht = vhpool.tile([P, NTOK], BF, name="htile", tag=f"vh_{ft}")
                nc.scalar.activation(ht, ps, gelu_f,
                                     bias=b1_sb[l][:, ft:ft + 1])
                h_sb.append(ht)
            # ---- y = W2 h (full-F PSUM accumulation, 2 waves of 4 dto) ----
            for half in range(2):
                ys = []
                for j in range(4):
                    yp = psum.tile([P, NTOK], F32, name="ps_y", tag="ps")
                    ys.append(yp)
                for ft in range(NFT):
                    w2_p = wpool.tile([P, 512], BF, name="w2_p", tag="w")
                    nc.sync.dma_start(w2_p, w2_d[l, half, ft])
                    for j in range(4):
                        mm2(ys[j], w2_p[:, j * P:(j + 1) * P], h_sb[ft],
                            start=(ft == 0), stop=(ft == NFT - 1))
                for j in range(4):
                    dto = half * 4 + j
                    nc.vector.scalar_tensor_tensor(
                        x_sb[dto], ys[j], b2_sb[l][:, dto:dto + 1],
                        xb_sb[dto], ALU.add, ALU.add)

        for l in range(L):
            attn_phase(l)
            ln(x_sb, g1_sb[l], be1_sb[l])
            ffn_phase(l)
            ln(x_sb, g2_sb[l], be2_sb[l])

        # ---- final LN + Wp (2 waves of 4 dto, dt-outer so matmuls start
        # as soon as the first normalized xb tile lands) ----
        # LNf consumes the LN2 output (xb), not the raw residual.
        ln(xb_sb, gf_sb, bf_sb)
        for half in range(2):
            wps, pss = [], []
            for j in range(4):
                dto = half * 4 + j
                wp_p = wpool.tile([P, NDT, P], BF, name="wp_p", tag="w")
                nc.sync.dma_start(
                    wp_p, wp_d[dto].rearrange("(t p) m -> p t m", p=P))
                wps.append(wp_p)
                pss.append(psum.tile([P, NTOK], F32, name="ps_wp", tag="ps"))
            for dt in range(NDT):
                for j in range(4):
                    mm2(pss[j], wps[j][:, dt, :], xb_sb[dt],
                        start=(dt == 0), stop=(dt == NDT - 1))
            for j in range(4):
                dto = half * 4 + j
                op = outpool.tile([P, NTOK], F32, name="outp", tag="out",
                                  bufs=2)
                nc.scalar.activation(op, pss[j], AF.Identity,
                                     bias=bp_sb[:, dto:dto + 1])
                nc.sync.dma_start(
                    out_d.ap()[:, dto * P:(dto + 1) * P, :].rearrange(
                        "b p s -> p b s"),
                    op.rearrange("p (b s) -> p b s", b=BPC))

    if reps == 1:
        body()
    else:
        with tc.For_i(0, reps, 1) as i:
            body(i)
    ctx.close()


# ======================= host side =======================

def _prep_core_inputs(inputs):
    """Build the 8 per-core input maps (weights shared, x/tau/delta sharded)."""
    import ml_dtypes
    bf = ml_dtypes.bfloat16
    f = np.float32
    x = np.asarray(inputs["x"], f)
    tau = np.asarray(inputs["tau"], f)
    delta = np.asarray(inputs["delta"], f)
    scale = 1.0 / np.sqrt(np.float32(DH))

    wq = np.asarray(inputs["Wq"], f)
    wk = np.asarray(inputs["Wk"], f)
    wv = np.asarray(inputs["Wv"], f)
    wo = np.asarray(inputs["Wo"], f)
    w1 = np.asarray(inputs["W1"], f)
    w2 = np.asarray(inputs["W2"], f)
    wp = np.asarray(inputs["Wp"], f)

    def hp_tiled(wt):  # [L, din, dout] -> [L, NHP, din, P]
        return np.ascontiguousarray(
            wt.reshape(L, D, NHP, P).transpose(0, 2, 1, 3)).astype(bf)

    wq_t = hp_tiled(wq.transpose(0, 2, 1))
    wk_t = hp_tiled(wk.transpose(0, 2, 1))
    wqk_t = np.ascontiguousarray(np.stack([wq_t, wk_t], axis=2))
    wo_t = hp_tiled(wo.transpose(0, 2, 1))
    wv_t = np.ascontiguousarray(wv.transpose(0, 2, 1)).astype(bf)
    # W1 [L, F, D] -> W1^T [L, D, F] -> [L, NFT, P(d), NDT, P(f)]
    w1_t = np.ascontiguousarray(
        w1.transpose(0, 2, 1).reshape(L, NDT, P, NFT, P)
        .transpose(0, 3, 2, 1, 4)).astype(bf)
    # W2 [L, D, F] -> W2^T [L, F, D] -> [L, 2, NFT, P(f), 512(dto cols)]
    w2_t = np.ascontiguousarray(
        w2.transpose(0, 2, 1).reshape(L, NFT, P, 2, 512)
        .transpose(0, 3, 1, 2, 4)).astype(bf)
    wp_t = np.ascontiguousarray(
        wp.transpose(1, 0).reshape(D, NDT, P).transpose(1, 0, 2)).astype(bf)

    shared = {
        "wqk_t": wqk_t, "wv_t": wv_t, "wo_t": wo_t,
        "w1_t": w1_t, "w2_t": w2_t, "wp_t": wp_t,
        "bv_bf": np.asarray(inputs["bv"], f).astype(bf),
    }

    def pcol(v):  # (n*P,) -> [P, n]
        v = np.asarray(v, f).reshape(-1, P)
        return v.T

    base_cols = []
    for l in range(L):
        for k in ("bq", "bk", "bo", "b2", "g1", "be1", "g2", "be2", "b1"):
            base_cols.append(pcol(inputs[k][l]))
    for k in ("gf", "bf", "bp"):
        base_cols.append(pcol(inputs[k]))

    in_maps = []
    for c in range(NCORES):
        bs = slice(c * BPC, (c + 1) * BPC)
        m = dict(shared)
        m["x_bf"] = np.ascontiguousarray(
            x[bs].transpose(0, 2, 1)).astype(bf)
        stau = np.tile((tau[bs] * scale).reshape(1, BPC), (P, 1))
        ed = np.exp(delta[bs] * scale).astype(f)          # [BPC, S]
        edc = np.ascontiguousarray(
            ed.reshape(BPC, NST, P).transpose(2, 0, 1).reshape(P, NDT))
        edr = np.repeat(edc[:, :, None], H, axis=2).reshape(P, NDT * H)
        m["pcols"] = np.ascontiguousarray(
            np.concatenate(base_cols + [stau, edc, edr], axis=1)).astype(f)
        in_maps.append(m)
    return in_maps


def run(inputs, reps=1):
    nc = _build(reps)
    in_maps = _prep_core_inputs(inputs)
    res = bass_utils.run_bass_kernel_spmd(nc, in_maps,
                                          core_ids=list(range(NCORES)))
    outs = [res.results[c]["out_fm"].transpose(0, 2, 1) for c in range(NCORES)]
    return np.ascontiguousarray(np.concatenate(outs, axis=0))


def kernel(**inputs) -> np.ndarray:
    return run(inputs, reps=1)
